# revision 8
# baseline (speedup 1.0000x reference)
"""Trainium2 Bass kernel for nn_FusedKQnA (sparse attention with learned
queries + depthwise stride-2 conv aggregation).

Math restructuring (vs the reference):
  - k is never materialized: qkT = x^T @ (Wk @ QW) with QW the block-diagonal
    arrangement of the scaled learned queries -> one (128->32) matmul.
  - The global max subtractions inside the two exp() calls cancel exactly
    between numerator and denominator, so they are dropped.
  - The 1024-channel depthwise conv never materializes.  With
    r = 1/sum_den (computed as exp(-ln(den)), same ACT table set) define
        gamma[t,h,ij] = sum_q kern[t,q*8+h] * r[q*8+h,ij] * cost[n_t(ij),q*8+h]
    Then out_pre[(h,c),ij] = sum_t gamma[t,h,ij] * v[n_t(ij),(h,c)]  (256 ch)
    and out = Wout @ out_pre.
  - gamma's q-contraction + broadcast over the 32 channels of each head is a
    single small PE matmul per (tap, channel-chunk) with a one-hot*kern
    stationary operand; the tap accumulation is PSUM accumulation through
    identity matmuls.

Sharding: pure data parallel over batch: 16 batches -> 8 cores x 2.
"""

import os
from contextlib import ExitStack

import numpy as np

import concourse.bass as bass
import concourse.mybir as mybir
import concourse.tile as tile
from concourse import bacc
from concourse.bass_utils import run_bass_kernel_spmd

# Problem constants (hardcoded per spec nn_FusedKQnA_1726576854813)
N_Q, N_HEADS, KSIZE, STRIDE, PADDING = 4, 4, 3, 2, 1
B, C, H, W = 16, 128, 56, 56
HC = C // N_HEADS            # 32 head channels
HP = N_HEADS * STRIDE        # 8 effective heads
CS = C * STRIDE              # 256
G = N_Q * HP                 # 32 kernel groups
HO, WO = H // STRIDE, W // STRIDE   # 28, 28
NCORES = 8
BPC = B // NCORES            # batches per core

TAPS = [(di, dj) for di in (-1, 0, 1) for dj in (-1, 0, 1)]
N_STRIPS = 2                 # output rows split into strips of 14 (392 px)
ROWS_PER_STRIP = HO // N_STRIPS

F32 = mybir.dt.float32
BF16 = mybir.dt.bfloat16

_BUILD_CACHE = {}


def _host_weights(Wk, Wv, Wout, q_param, attn_scale, rpb_table):
    """Precompute all small weight tensors on the host."""
    q = q_param.reshape(N_Q, HP, HC).astype(np.float64) * (HC ** -0.5)
    QW = np.zeros((CS, G), np.float64)
    for qi in range(N_Q):
        for h in range(HP):
            QW[h * HC:(h + 1) * HC, qi * HP + h] = q[qi, h]
    wkq = (Wk.astype(np.float64) @ QW).astype(np.float32)        # (128, 32)

    rpb_exp = np.exp(rpb_table.astype(np.float64))               # (9, 32)
    kern_num = (rpb_exp * attn_scale.astype(np.float64))         # (9, 32)

    # denominator conv kernels as per-partition scalar columns: (32, 9)
    denk = rpb_exp.T.astype(np.float32).copy()                   # (G, 9)

    # gamma-broadcast stationary operands: kmat[t, ch] has shape (32, 128)
    # kmat[t,ch][g, m] = kern_num[t, g] if g % HP == ch*4 + m//HC else 0
    kmat = np.zeros((KSIZE * KSIZE, 2, G, 128), np.float32)
    for t in range(KSIZE * KSIZE):
        for ch in range(2):
            for g in range(G):
                h = g % HP
                if h // 4 == ch:
                    m0 = (h % 4) * HC
                    kmat[t, ch, g, m0:m0 + HC] = kern_num[t, g]

    woutT = np.ascontiguousarray(Wout.T.astype(np.float32))      # (256, 256) lhsT
    ident = np.eye(128, dtype=np.float32)
    return dict(wkq=wkq, denk=denk, kmat=kmat, woutT=woutT, ident=ident,
                wv=np.ascontiguousarray(Wv.astype(np.float32)))


def _build_program():
    """Build the Bass/Tile program once. Returns (nc, input_names)."""
    nc = bacc.Bacc("TRN2", target_bir_lowering=False, debug=False,
                   enable_asserts=False, num_devices=NCORES)

    x_d = nc.dram_tensor("x", [BPC, C, H, W], F32, kind="ExternalInput").ap()
    wkq_d = nc.dram_tensor("wkq", [C, G], F32, kind="ExternalInput").ap()
    wv_d = nc.dram_tensor("wv", [C, CS], F32, kind="ExternalInput").ap()
    denk_d = nc.dram_tensor("denk", [G, 9], F32, kind="ExternalInput").ap()
    kmat_d = nc.dram_tensor("kmat", [9, 2, G, 128], F32, kind="ExternalInput").ap()
    woutT_d = nc.dram_tensor("woutT", [CS, CS], F32, kind="ExternalInput").ap()
    ident_d = nc.dram_tensor("ident", [128, 128], F32, kind="ExternalInput").ap()
    out_d = nc.dram_tensor("out", [BPC, CS, HO, WO], F32, kind="ExternalOutput").ap()

    with tile.TileContext(nc) as tc, ExitStack() as ctx:
        _kernel_body(ctx, tc, out_d, x_d, wkq_d, wv_d, denk_d, kmat_d,
                     woutT_d, ident_d)

    nc.compile()
    return nc


def _kernel_body(ctx, tc, out_d, x_d, wkq_d, wv_d, denk_d, kmat_d,
                 woutT_d, ident_d):
    nc = tc.nc
    PH, PW = H + 2, W + 2        # padded plane 58 x 58

    consts = ctx.enter_context(tc.tile_pool(name="consts", bufs=1))
    planes = ctx.enter_context(tc.tile_pool(name="planes", bufs=1))
    xpool = ctx.enter_context(tc.tile_pool(name="xpool", bufs=2))
    small = ctx.enter_context(tc.tile_pool(name="small", bufs=1))
    rcpool = ctx.enter_context(tc.tile_pool(name="rcpool", bufs=1))
    prod_pool = ctx.enter_context(tc.tile_pool(name="prod", bufs=4))
    opre_pool = ctx.enter_context(tc.tile_pool(name="opre", bufs=1))
    outs_pool = ctx.enter_context(tc.tile_pool(name="outs", bufs=4))

    ps = ctx.enter_context(tc.tile_pool(name="ps", bufs=2, space="PSUM"))

    # ---- constants into SBUF ----
    wkq_sb = consts.tile([C, G], F32)
    nc.sync.dma_start(out=wkq_sb, in_=wkq_d)
    wv_sb = consts.tile([C, CS], F32)
    nc.sync.dma_start(out=wv_sb, in_=wv_d)
    denk_sb = consts.tile([G, 9], F32)
    nc.sync.dma_start(out=denk_sb, in_=denk_d)
    kmat_sb = consts.tile([G, 9, 2, 128], BF16)
    nc.gpsimd.dma_start(out=kmat_sb, in_=kmat_d.transpose([2, 0, 1, 3]))
    woutT_sb = consts.tile([128, 2, CS], F32)
    nc.sync.dma_start(out=woutT_sb,
                      in_=woutT_d.rearrange("(kc k) m -> k kc m", kc=2))
    ident_sb = consts.tile([128, 128], F32)
    nc.sync.dma_start(out=ident_sb, in_=ident_d)

    # ---- persistent padded planes (zero borders set once) ----
    cost_pl = [planes.tile([G, PH, PW], F32, tag=f"cost{b}", name=f"cost_pl{b}") for b in range(BPC)]
    v_pl = [[planes.tile([128, PH, PW], F32, tag=f"v{b}_{chn}", name=f"v_pl{b}_{chn}") for chn in range(2)]
            for b in range(BPC)]
    for b in range(BPC):
        nc.vector.memset(cost_pl[b], 0.0)
        nc.vector.memset(v_pl[b][0], 0.0)
        nc.vector.memset(v_pl[b][1], 0.0)

    n_row_tiles = 7          # 56 rows in tiles of 8 -> matmul N=448
    RT = H // n_row_tiles    # 8 rows per tile

    for b in range(BPC):
        x_sb = xpool.tile([C, H, W], F32)
        nc.sync.dma_start(out=x_sb, in_=x_d[b])

        # ---- qkT + exp -> cost plane ----
        for rt in range(n_row_tiles):
            qk_ps = ps.tile([G, RT, W], F32, tag="mm", bufs=3, name="qk_ps")
            nc.tensor.matmul(qk_ps, wkq_sb, x_sb[:, rt * RT:(rt + 1) * RT, :],
                             start=True, stop=True)
            nc.scalar.activation(
                out=cost_pl[b][:, 1 + rt * RT:1 + (rt + 1) * RT, 1:1 + W],
                in_=qk_ps, func=mybir.ActivationFunctionType.Exp)

        # ---- v matmuls -> v planes ----
        for chn in range(2):
            for rt in range(n_row_tiles):
                v_ps = ps.tile([128, RT, W], F32, tag="mm", bufs=3, name="v_ps")
                nc.tensor.matmul(v_ps, wv_sb[:, chn * 128:(chn + 1) * 128],
                                 x_sb[:, rt * RT:(rt + 1) * RT, :],
                                 start=True, stop=True)
                nc.scalar.copy(
                    out=v_pl[b][chn][:, 1 + rt * RT:1 + (rt + 1) * RT, 1:1 + W],
                    in_=v_ps)

        # ---- denominator conv + r = exp(-ln(den)) ----
        den = small.tile([G, HO, WO], F32)
        for t, (di, dj) in enumerate(TAPS):
            cv = cost_pl[b][:, 1 + di:1 + di + 2 * HO:2, 1 + dj:1 + dj + 2 * WO:2]
            if t == 0:
                nc.vector.tensor_scalar_mul(den, cv, denk_sb[:, 0:1])
            else:
                nc.vector.scalar_tensor_tensor(
                    out=den, in0=cv, scalar=denk_sb[:, t:t + 1], in1=den,
                    op0=mybir.AluOpType.mult, op1=mybir.AluOpType.add)
        lden = small.tile([G, HO, WO], F32)
        nc.scalar.activation(out=lden, in_=den,
                             func=mybir.ActivationFunctionType.Ln)
        r_sb = small.tile([G, HO, WO], F32)
        nc.scalar.activation(out=r_sb, in_=lden, scale=-1.0,
                             func=mybir.ActivationFunctionType.Exp)

        # ---- rc[t] = cost_t * r ----
        rc_st = [rcpool.tile([G, HO, WO], BF16, tag=f"rc{t}", name=f"rc_st{t}")
                 for t in range(9)]
        for t, (di, dj) in enumerate(TAPS):
            cv = cost_pl[b][:, 1 + di:1 + di + 2 * HO:2, 1 + dj:1 + dj + 2 * WO:2]
            nc.vector.tensor_mul(rc_st[t], cv, r_sb)

        # ---- per (chunk, strip): gamma matmul, product, identity-accum ----
        opre_sb = {}
        for chn in range(2):
            for s in range(N_STRIPS):
                acc_ps = ps.tile([128, ROWS_PER_STRIP, WO], F32,
                                 tag="acc", bufs=2, name="acc_ps")
                r0 = s * ROWS_PER_STRIP
                for t, (di, dj) in enumerate(TAPS):
                    gam_ps = ps.tile([128, ROWS_PER_STRIP, WO], F32, tag="gam", bufs=2, name="gam_ps")
                    nc.tensor.matmul(
                        gam_ps, kmat_sb[:, t, chn, :],
                        rc_st[t][:, r0:r0 + ROWS_PER_STRIP, :],
                        start=True, stop=True)
                    p_sb = prod_pool.tile([128, ROWS_PER_STRIP, WO], F32)
                    vv = v_pl[b][chn][:,
                                      1 + di + 2 * r0:1 + di + 2 * (r0 + ROWS_PER_STRIP):2,
                                      1 + dj:1 + dj + 2 * WO:2]
                    nc.vector.tensor_mul(p_sb, gam_ps, vv)
                    nc.tensor.matmul(acc_ps, ident_sb, p_sb,
                                     start=(t == 0), stop=(t == 8))
                o_sb = opre_pool.tile([128, ROWS_PER_STRIP, WO], F32,
                                      tag=f"opre{chn}_{s}")
                nc.scalar.copy(out=o_sb, in_=acc_ps)
                opre_sb[(chn, s)] = o_sb

        # ---- Wout projection + store ----
        for mo in range(2):
            for s in range(N_STRIPS):
                out_ps = ps.tile([128, ROWS_PER_STRIP, WO], F32, tag="mm", bufs=3, name="out_ps")
                for kc in range(2):
                    nc.tensor.matmul(out_ps,
                                     woutT_sb[:, kc, mo * 128:(mo + 1) * 128],
                                     opre_sb[(kc, s)],
                                     start=(kc == 0), stop=(kc == 1))
                o_final = outs_pool.tile([128, ROWS_PER_STRIP, WO], F32)
                nc.scalar.copy(out=o_final, in_=out_ps)
                nc.sync.dma_start(
                    out=out_d[b, mo * 128:(mo + 1) * 128,
                              s * ROWS_PER_STRIP:(s + 1) * ROWS_PER_STRIP, :],
                    in_=o_final)


def _install_ntff_shim():
    """bass_utils expects antenv.axon_hooks (absent in this checkout); shim it
    with the ctypes NTFF hook from trn_agent_boot so trace=True works."""
    import sys
    import types
    try:
        from antenv.axon_hooks import get_axon_ntff_profile_hook  # noqa: F401
        return
    except ImportError:
        pass
    try:
        from trn_agent_boot.trn_boot import _ntff_profile_via_ctypes
        hook = _ntff_profile_via_ctypes("/opt/axon/libaxon_pjrt.so")
    except Exception:
        hook = None
    mod = types.ModuleType("antenv.axon_hooks")
    mod._hook = hook
    mod.get_axon_ntff_profile_hook = lambda: mod._hook
    mod.set_axon_ntff_profile_hook = lambda h: setattr(mod, "_hook", h)
    sys.modules["antenv.axon_hooks"] = mod


def _get_program():
    if "nc" not in _BUILD_CACHE:
        _BUILD_CACHE["nc"] = _build_program()
    return _BUILD_CACHE["nc"]


def kernel(x, Wk, Wv, Wout, q_param, attn_scale, rpb_table):
    x = np.ascontiguousarray(np.asarray(x, dtype=np.float32))
    wts = _host_weights(np.asarray(Wk), np.asarray(Wv), np.asarray(Wout),
                        np.asarray(q_param), np.asarray(attn_scale),
                        np.asarray(rpb_table))
    nc = _get_program()

    in_maps = []
    for c in range(NCORES):
        in_maps.append({
            "x": np.ascontiguousarray(x[c * BPC:(c + 1) * BPC]),
            "wkq": wts["wkq"], "wv": wts["wv"], "denk": wts["denk"],
            "kmat": wts["kmat"], "woutT": wts["woutT"], "ident": wts["ident"],
        })

    trace = bool(int(os.environ.get("KERNEL_TRACE", "0")))
    if trace:
        _install_ntff_shim()
    res = run_bass_kernel_spmd(nc, in_maps, core_ids=list(range(NCORES)),
                               trace=trace)
    _BUILD_CACHE["last_results"] = res

    out = np.empty((B, CS, HO, WO), np.float32)
    for c in range(NCORES):
        out[c * BPC:(c + 1) * BPC] = res.results[c]["out"]
    return out


# revision 12
# speedup vs baseline: 1.2458x; 1.2458x over previous
"""Trainium2 Bass kernel for nn_FusedKQnA (sparse attention with learned
queries + depthwise stride-2 conv aggregation).

Math restructuring (vs the reference):
  - k is never materialized: qkT = x^T @ (Wk @ QW) with QW the block-diagonal
    arrangement of the scaled learned queries -> one (128->32) matmul.
  - The global max subtractions inside the two exp() calls cancel exactly
    between numerator and denominator, so they are dropped.
  - The 1024-channel depthwise conv never materializes.  With
    r = 1/sum_den (computed as exp(-ln(den)), same ACT table set) define
        gamma[t,h,ij] = sum_q kern[t,q*8+h] * r[q*8+h,ij] * cost[n_t(ij),q*8+h]
    Then out_pre[(h,c),ij] = sum_t gamma[t,h,ij] * v[n_t(ij),(h,c)]  (256 ch)
    and out = Wout @ out_pre.
  - gamma's q-contraction + broadcast over the 32 channels of each head is a
    single small PE matmul per (tap, channel-chunk) with a one-hot*kern
    stationary operand; the tap accumulation is PSUM accumulation through
    identity matmuls.

Sharding: pure data parallel over batch: 16 batches -> 8 cores x 2.
"""

import os
from contextlib import ExitStack

import numpy as np

import concourse.bass as bass
import concourse.mybir as mybir
import concourse.tile as tile
from concourse import bacc
from concourse.bass_utils import run_bass_kernel_spmd

# Problem constants (hardcoded per spec nn_FusedKQnA_1726576854813)
N_Q, N_HEADS, KSIZE, STRIDE, PADDING = 4, 4, 3, 2, 1
B, C, H, W = 16, 128, 56, 56
HC = C // N_HEADS            # 32 head channels
HP = N_HEADS * STRIDE        # 8 effective heads
CS = C * STRIDE              # 256
G = N_Q * HP                 # 32 kernel groups
HO, WO = H // STRIDE, W // STRIDE   # 28, 28
NCORES = 8
BPC = B // NCORES            # batches per core

TAPS = [(di, dj) for di in (-1, 0, 1) for dj in (-1, 0, 1)]
N_STRIPS = 2                 # output rows split into strips of 14 (392 px)
ROWS_PER_STRIP = HO // N_STRIPS

F32 = mybir.dt.float32
BF16 = mybir.dt.bfloat16

_BUILD_CACHE = {}


def _host_weights(Wk, Wv, Wout, q_param, attn_scale, rpb_table):
    """Precompute all small weight tensors on the host."""
    q = q_param.reshape(N_Q, HP, HC).astype(np.float64) * (HC ** -0.5)
    QW = np.zeros((CS, G), np.float64)
    for qi in range(N_Q):
        for h in range(HP):
            QW[h * HC:(h + 1) * HC, qi * HP + h] = q[qi, h]
    wkq = (Wk.astype(np.float64) @ QW).astype(np.float32)        # (128, 32)

    rpb_exp = np.exp(rpb_table.astype(np.float64))               # (9, 32)
    kern_num = (rpb_exp * attn_scale.astype(np.float64))         # (9, 32)

    # denominator conv kernels as per-partition scalar columns: (32, 9)
    denk = rpb_exp.T.astype(np.float32).copy()                   # (G, 9)

    # gamma-broadcast stationary operands: kmat[t, ch] has shape (32, 128)
    # kmat[t,ch][g, m] = kern_num[t, g] if g % HP == ch*4 + m//HC else 0
    kmat = np.zeros((KSIZE * KSIZE, 2, G, 128), np.float32)
    for t in range(KSIZE * KSIZE):
        for ch in range(2):
            for g in range(G):
                h = g % HP
                if h // 4 == ch:
                    m0 = (h % 4) * HC
                    kmat[t, ch, g, m0:m0 + HC] = kern_num[t, g]

    woutT = np.ascontiguousarray(Wout.T.astype(np.float32))      # (256, 256) lhsT
    ident = np.eye(128, dtype=np.float32)
    return dict(wkq=wkq, denk=denk, kmat=kmat, woutT=woutT, ident=ident,
                wv=np.ascontiguousarray(Wv.astype(np.float32)))


def _build_program():
    """Build the Bass/Tile program once. Returns (nc, input_names)."""
    nc = bacc.Bacc("TRN2", target_bir_lowering=False, debug=False,
                   enable_asserts=False, num_devices=NCORES)

    x_d = nc.dram_tensor("x", [BPC, C, H, W], F32, kind="ExternalInput").ap()
    wkq_d = nc.dram_tensor("wkq", [C, G], F32, kind="ExternalInput").ap()
    wv_d = nc.dram_tensor("wv", [C, CS], F32, kind="ExternalInput").ap()
    denk_d = nc.dram_tensor("denk", [G, 9], F32, kind="ExternalInput").ap()
    kmat_d = nc.dram_tensor("kmat", [9, 2, G, 128], F32, kind="ExternalInput").ap()
    woutT_d = nc.dram_tensor("woutT", [CS, CS], F32, kind="ExternalInput").ap()
    ident_d = nc.dram_tensor("ident", [128, 128], F32, kind="ExternalInput").ap()
    out_d = nc.dram_tensor("out", [BPC, CS, HO, WO], F32, kind="ExternalOutput").ap()

    with tile.TileContext(nc) as tc, ExitStack() as ctx:
        _kernel_body(ctx, tc, out_d, x_d, wkq_d, wv_d, denk_d, kmat_d,
                     woutT_d, ident_d)

    _pin_act_tables()
    nc.compile()
    return nc


def _pin_act_tables():
    """Force one ACT table set (natural_log_exp_and_others) for Exp+Ln so the
    scheduler doesn't thrash table loads between them."""
    from concourse import hw_specs
    import concourse.bacc as bacc_mod
    if getattr(bacc_mod, "_act_tables_pinned", False):
        return
    orig = hw_specs.get_activation_tables

    def patched(arch):
        tabs = dict(orig(arch))
        keep = "natural_log_exp_and_others"
        for name in list(tabs):
            if name == keep:
                continue
            fns = tabs[name]
            if any(str(f).endswith((".Exp", ".Ln")) for f in fns):
                tabs[name] = type(fns)()
        return tabs

    bacc_mod.get_activation_tables = patched
    bacc_mod._act_tables_pinned = True


def _kernel_body(ctx, tc, out_d, x_d, wkq_d, wv_d, denk_d, kmat_d,
                 woutT_d, ident_d):
    nc = tc.nc
    PH, PW = H + 2, W + 2        # padded plane 58 x 58

    consts = ctx.enter_context(tc.tile_pool(name="consts", bufs=1))
    planes = ctx.enter_context(tc.tile_pool(name="planes", bufs=1))
    xpool = ctx.enter_context(tc.tile_pool(name="xpool", bufs=2))
    small = ctx.enter_context(tc.tile_pool(name="small", bufs=1))
    rcpool = ctx.enter_context(tc.tile_pool(name="rcpool", bufs=1))
    prod_pool = ctx.enter_context(tc.tile_pool(name="prod", bufs=4))
    opre_pool = ctx.enter_context(tc.tile_pool(name="opre", bufs=1))
    outs_pool = ctx.enter_context(tc.tile_pool(name="outs", bufs=4))

    ps = ctx.enter_context(tc.tile_pool(name="ps", bufs=2, space="PSUM"))

    # ---- constants into SBUF ----
    wkq_sb = consts.tile([C, G], F32)
    nc.sync.dma_start(out=wkq_sb, in_=wkq_d)
    wv_sb = consts.tile([C, CS], F32)
    nc.sync.dma_start(out=wv_sb, in_=wv_d)
    denk_sb = consts.tile([G, 9], F32)
    nc.sync.dma_start(out=denk_sb, in_=denk_d)
    kmat_sb = consts.tile([G, 9, 2, 128], BF16)
    nc.gpsimd.dma_start(out=kmat_sb, in_=kmat_d.transpose([2, 0, 1, 3]))
    woutT_sb = consts.tile([128, 2, CS], BF16)
    nc.gpsimd.dma_start(out=woutT_sb,
                        in_=woutT_d.rearrange("(kc k) m -> k kc m", kc=2))
    ident_sb = consts.tile([128, 128], BF16)
    nc.gpsimd.dma_start(out=ident_sb, in_=ident_d)

    # ---- persistent padded planes (zero borders set once) ----
    cost_pl = [planes.tile([G, PH, PW], F32, tag=f"cost{b}", name=f"cost_pl{b}") for b in range(BPC)]
    v_pl = [[planes.tile([128, PH, PW], F32, tag=f"v{b}_{chn}", name=f"v_pl{b}_{chn}") for chn in range(2)]
            for b in range(BPC)]
    for pl in [cost_pl[b] for b in range(BPC)] + [v_pl[b][c] for b in range(BPC)
                                                 for c in range(2)]:
        # interior is fully overwritten every batch; only borders must be 0
        nc.gpsimd.memset(pl[:, 0, :], 0.0)
        nc.gpsimd.memset(pl[:, PH - 1, :], 0.0)
        nc.gpsimd.memset(pl[:, 1:PH - 1, 0], 0.0)
        nc.gpsimd.memset(pl[:, 1:PH - 1, PW - 1], 0.0)

    n_row_tiles = 7          # 56 rows in tiles of 8 -> matmul N=448
    RT = H // n_row_tiles    # 8 rows per tile

    for b in range(BPC):
        x_sb = xpool.tile([C, H, W], F32)
        nc.sync.dma_start(out=x_sb, in_=x_d[b])

        # ---- qkT + exp -> cost plane ----
        for rt in range(n_row_tiles):
            qk_ps = ps.tile([G, RT, W], F32, tag="mm", bufs=2, name="qk_ps")
            nc.tensor.matmul(qk_ps, wkq_sb, x_sb[:, rt * RT:(rt + 1) * RT, :],
                             start=True, stop=True)
            nc.scalar.activation(
                out=cost_pl[b][:, 1 + rt * RT:1 + (rt + 1) * RT, 1:1 + W],
                in_=qk_ps, func=mybir.ActivationFunctionType.Exp)

        # ---- v matmuls -> v planes ----
        for chn in range(2):
            for rt in range(n_row_tiles):
                v_ps = ps.tile([128, RT, W], F32, tag="mm", bufs=2, name="v_ps")
                nc.tensor.matmul(v_ps, wv_sb[:, chn * 128:(chn + 1) * 128],
                                 x_sb[:, rt * RT:(rt + 1) * RT, :],
                                 start=True, stop=True)
                nc.scalar.copy(
                    out=v_pl[b][chn][:, 1 + rt * RT:1 + (rt + 1) * RT, 1:1 + W],
                    in_=v_ps)

        # ---- denominator conv + r = exp(-ln(den)) ----
        den = small.tile([G, HO, WO], F32)
        for t, (di, dj) in enumerate(TAPS):
            cv = cost_pl[b][:, 1 + di:1 + di + 2 * HO:2, 1 + dj:1 + dj + 2 * WO:2]
            if t == 0:
                nc.vector.tensor_scalar_mul(den, cv, denk_sb[:, 0:1])
            else:
                nc.vector.scalar_tensor_tensor(
                    out=den, in0=cv, scalar=denk_sb[:, t:t + 1], in1=den,
                    op0=mybir.AluOpType.mult, op1=mybir.AluOpType.add)
        lden = small.tile([G, HO, WO], F32)
        nc.scalar.activation(out=lden, in_=den,
                             func=mybir.ActivationFunctionType.Ln)
        r_sb = small.tile([G, HO, WO], F32)
        nc.scalar.activation(out=r_sb, in_=lden, scale=-1.0,
                             func=mybir.ActivationFunctionType.Exp)

        # ---- rc[t] = cost_t * r ----
        rc_st = [rcpool.tile([G, HO, WO], BF16, tag=f"rc{t}", name=f"rc_st{t}")
                 for t in range(9)]
        for t, (di, dj) in enumerate(TAPS):
            cv = cost_pl[b][:, 1 + di:1 + di + 2 * HO:2, 1 + dj:1 + dj + 2 * WO:2]
            nc.vector.tensor_mul(rc_st[t], cv, r_sb)

        # ---- per chunk: gamma matmuls (2 strips/psum tile), fused product,
        #      identity-matmul tap accumulation ----
        opre_sb = {}
        for chn in range(2):
            accs = [ps.tile([128, ROWS_PER_STRIP, WO], F32, tag="acc", bufs=2,
                            name=f"acc_ps{s}") for s in range(N_STRIPS)]
            for t, (di, dj) in enumerate(TAPS):
                # one 2-bank psum tile holds both strips of gamma_bc
                gam_ps = ps.tile([128, 2, 512], F32, tag="gam", bufs=2,
                                 name="gam_ps")
                for s in range(N_STRIPS):
                    r0 = s * ROWS_PER_STRIP
                    gv = gam_ps[:, s, :ROWS_PER_STRIP * WO].rearrange(
                        "p (a c) -> p a c", a=ROWS_PER_STRIP)
                    nc.tensor.matmul(gv, kmat_sb[:, t, chn, :],
                                     rc_st[t][:, r0:r0 + ROWS_PER_STRIP, :],
                                     start=True, stop=True)
                # one full-size product op (reads gamma straight from psum)
                p_sb = prod_pool.tile([128, 2, ROWS_PER_STRIP, WO], BF16)
                gfull = gam_ps[:, :, :ROWS_PER_STRIP * WO].rearrange(
                    "p s (a c) -> p s a c", a=ROWS_PER_STRIP)
                vv = v_pl[b][chn][:, 1 + di:1 + di + 2 * HO:2,
                                  1 + dj:1 + dj + 2 * WO:2].rearrange(
                    "p (s a) c -> p s a c", s=2)
                nc.vector.tensor_mul(p_sb, gfull, vv)
                for s in range(N_STRIPS):
                    nc.tensor.matmul(accs[s], ident_sb, p_sb[:, s],
                                     start=(t == 0), stop=(t == 8))
            for s in range(N_STRIPS):
                o_sb = opre_pool.tile([128, ROWS_PER_STRIP, WO], BF16,
                                      tag=f"opre{chn}_{s}", name=f"opre{chn}{s}")
                nc.scalar.copy(out=o_sb, in_=accs[s])
                opre_sb[(chn, s)] = o_sb

        # ---- Wout projection + store ----
        for mo in range(2):
            for s in range(N_STRIPS):
                out_ps = ps.tile([128, ROWS_PER_STRIP, WO], F32, tag="mm", bufs=2, name="out_ps")
                for kc in range(2):
                    nc.tensor.matmul(out_ps,
                                     woutT_sb[:, kc, mo * 128:(mo + 1) * 128],
                                     opre_sb[(kc, s)],
                                     start=(kc == 0), stop=(kc == 1))
                o_final = outs_pool.tile([128, ROWS_PER_STRIP, WO], F32)
                nc.scalar.copy(out=o_final, in_=out_ps)
                nc.sync.dma_start(
                    out=out_d[b, mo * 128:(mo + 1) * 128,
                              s * ROWS_PER_STRIP:(s + 1) * ROWS_PER_STRIP, :],
                    in_=o_final)


def _install_ntff_shim():
    """bass_utils expects antenv.axon_hooks (absent in this checkout); shim it
    with the ctypes NTFF hook from trn_agent_boot so trace=True works."""
    import sys
    import types
    try:
        from antenv.axon_hooks import get_axon_ntff_profile_hook  # noqa: F401
        return
    except ImportError:
        pass
    try:
        from trn_agent_boot.trn_boot import _ntff_profile_via_ctypes
        hook = _ntff_profile_via_ctypes("/opt/axon/libaxon_pjrt.so")
    except Exception:
        hook = None
    mod = types.ModuleType("antenv.axon_hooks")
    mod._hook = hook
    mod.get_axon_ntff_profile_hook = lambda: mod._hook
    mod.set_axon_ntff_profile_hook = lambda h: setattr(mod, "_hook", h)
    sys.modules["antenv.axon_hooks"] = mod


def _get_program():
    if "nc" not in _BUILD_CACHE:
        _BUILD_CACHE["nc"] = _build_program()
    return _BUILD_CACHE["nc"]


def kernel(x, Wk, Wv, Wout, q_param, attn_scale, rpb_table):
    x = np.ascontiguousarray(np.asarray(x, dtype=np.float32))
    wts = _host_weights(np.asarray(Wk), np.asarray(Wv), np.asarray(Wout),
                        np.asarray(q_param), np.asarray(attn_scale),
                        np.asarray(rpb_table))
    nc = _get_program()

    in_maps = []
    for c in range(NCORES):
        in_maps.append({
            "x": np.ascontiguousarray(x[c * BPC:(c + 1) * BPC]),
            "wkq": wts["wkq"], "wv": wts["wv"], "denk": wts["denk"],
            "kmat": wts["kmat"], "woutT": wts["woutT"], "ident": wts["ident"],
        })

    trace = bool(int(os.environ.get("KERNEL_TRACE", "0")))
    if trace:
        _install_ntff_shim()
    res = run_bass_kernel_spmd(nc, in_maps, core_ids=list(range(NCORES)),
                               trace=trace)
    _BUILD_CACHE["last_results"] = res

    out = np.empty((B, CS, HO, WO), np.float32)
    for c in range(NCORES):
        out[c * BPC:(c + 1) * BPC] = res.results[c]["out"]
    return out


# revision 14
# speedup vs baseline: 1.3979x; 1.1221x over previous
"""Trainium2 Bass kernel for nn_FusedKQnA (sparse attention with learned
queries + depthwise stride-2 conv aggregation).

Math restructuring (vs the reference):
  - k is never materialized: qkT = x^T @ (Wk @ QW) with QW the block-diagonal
    arrangement of the scaled learned queries -> one (128->32) matmul.
  - The global max subtractions inside the two exp() calls cancel exactly
    between numerator and denominator, so they are dropped.
  - The 1024-channel depthwise conv never materializes.  With
    r = 1/sum_den (computed as exp(-ln(den)), same ACT table set) define
        gamma[t,h,ij] = sum_q kern[t,q*8+h] * r[q*8+h,ij] * cost[n_t(ij),q*8+h]
    Then out_pre[(h,c),ij] = sum_t gamma[t,h,ij] * v[n_t(ij),(h,c)]  (256 ch)
    and out = Wout @ out_pre.
  - gamma's q-contraction + broadcast over the 32 channels of each head is a
    single small PE matmul per (tap, channel-chunk) with a one-hot*kern
    stationary operand; the tap accumulation is PSUM accumulation through
    identity matmuls.

Sharding: pure data parallel over batch: 16 batches -> 8 cores x 2.
"""

import os
from contextlib import ExitStack

import numpy as np

import concourse.bass as bass
import concourse.mybir as mybir
import concourse.tile as tile
from concourse import bacc
from concourse.bass_utils import run_bass_kernel_spmd

# Problem constants (hardcoded per spec nn_FusedKQnA_1726576854813)
N_Q, N_HEADS, KSIZE, STRIDE, PADDING = 4, 4, 3, 2, 1
B, C, H, W = 16, 128, 56, 56
HC = C // N_HEADS            # 32 head channels
HP = N_HEADS * STRIDE        # 8 effective heads
CS = C * STRIDE              # 256
G = N_Q * HP                 # 32 kernel groups
HO, WO = H // STRIDE, W // STRIDE   # 28, 28
NCORES = 8
BPC = B // NCORES            # batches per core

TAPS = [(di, dj) for di in (-1, 0, 1) for dj in (-1, 0, 1)]
N_STRIPS = 2                 # output rows split into strips of 14 (392 px)
ROWS_PER_STRIP = HO // N_STRIPS

F32 = mybir.dt.float32
BF16 = mybir.dt.bfloat16

_BUILD_CACHE = {}


def _host_weights(Wk, Wv, Wout, q_param, attn_scale, rpb_table):
    """Precompute all small weight tensors on the host."""
    q = q_param.reshape(N_Q, HP, HC).astype(np.float64) * (HC ** -0.5)
    QW = np.zeros((CS, G), np.float64)
    for qi in range(N_Q):
        for h in range(HP):
            QW[h * HC:(h + 1) * HC, qi * HP + h] = q[qi, h]
    wkq = (Wk.astype(np.float64) @ QW).astype(np.float32)        # (128, 32)

    rpb_exp = np.exp(rpb_table.astype(np.float64))               # (9, 32)
    kern_num = (rpb_exp * attn_scale.astype(np.float64))         # (9, 32)

    # denominator conv kernels as per-partition scalar columns: (32, 9)
    denk = rpb_exp.T.astype(np.float32).copy()                   # (G, 9)

    # gamma-broadcast stationary operands: kmat[t, ch] has shape (32, 128)
    # kmat[t,ch][g, m] = kern_num[t, g] if g % HP == ch*4 + m//HC else 0
    kmat = np.zeros((KSIZE * KSIZE, 2, G, 128), np.float32)
    for t in range(KSIZE * KSIZE):
        for ch in range(2):
            for g in range(G):
                h = g % HP
                if h // 4 == ch:
                    m0 = (h % 4) * HC
                    kmat[t, ch, g, m0:m0 + HC] = kern_num[t, g]

    woutT = np.ascontiguousarray(Wout.T.astype(np.float32))      # (256, 256) lhsT
    ident = np.eye(128, dtype=np.float32)
    return dict(wkq=wkq, denk=denk, kmat=kmat, woutT=woutT, ident=ident,
                wv=np.ascontiguousarray(Wv.astype(np.float32)))


def _build_program():
    """Build the Bass/Tile program once. Returns (nc, input_names)."""
    nc = bacc.Bacc("TRN2", target_bir_lowering=False, debug=False,
                   enable_asserts=False, num_devices=NCORES)

    x_d = nc.dram_tensor("x", [BPC, C, H, W], F32, kind="ExternalInput").ap()
    wkq_d = nc.dram_tensor("wkq", [C, G], F32, kind="ExternalInput").ap()
    wv_d = nc.dram_tensor("wv", [C, CS], F32, kind="ExternalInput").ap()
    denk_d = nc.dram_tensor("denk", [G, 9], F32, kind="ExternalInput").ap()
    kmat_d = nc.dram_tensor("kmat", [9, 2, G, 128], F32, kind="ExternalInput").ap()
    woutT_d = nc.dram_tensor("woutT", [CS, CS], F32, kind="ExternalInput").ap()
    ident_d = nc.dram_tensor("ident", [128, 128], F32, kind="ExternalInput").ap()
    out_d = nc.dram_tensor("out", [BPC, CS, HO, WO], F32, kind="ExternalOutput").ap()

    with tile.TileContext(nc) as tc, ExitStack() as ctx:
        _kernel_body(ctx, tc, out_d, x_d, wkq_d, wv_d, denk_d, kmat_d,
                     woutT_d, ident_d)

    _pin_act_tables()
    nc.compile()
    return nc


def _pin_act_tables():
    """Force one ACT table set (natural_log_exp_and_others) for Exp+Ln so the
    scheduler doesn't thrash table loads between them."""
    from concourse import hw_specs
    import concourse.bacc as bacc_mod
    if getattr(bacc_mod, "_act_tables_pinned", False):
        return
    orig = hw_specs.get_activation_tables

    def patched(arch):
        tabs = dict(orig(arch))
        keep = "natural_log_exp_and_others"
        for name in list(tabs):
            if name == keep:
                continue
            fns = tabs[name]
            if any(str(f).endswith((".Exp", ".Ln")) for f in fns):
                tabs[name] = type(fns)()
        return tabs

    bacc_mod.get_activation_tables = patched
    bacc_mod._act_tables_pinned = True


def _kernel_body(ctx, tc, out_d, x_d, wkq_d, wv_d, denk_d, kmat_d,
                 woutT_d, ident_d):
    nc = tc.nc
    PH, PW = H + 2, W + 2        # padded plane 58 x 58

    consts = ctx.enter_context(tc.tile_pool(name="consts", bufs=1))
    planes = ctx.enter_context(tc.tile_pool(name="planes", bufs=1))
    xpool = ctx.enter_context(tc.tile_pool(name="xpool", bufs=2))
    small = ctx.enter_context(tc.tile_pool(name="small", bufs=2))
    rcpool = ctx.enter_context(tc.tile_pool(name="rcpool", bufs=2))
    prod_pool = ctx.enter_context(tc.tile_pool(name="prod", bufs=4))
    opre_pool = ctx.enter_context(tc.tile_pool(name="opre", bufs=2))
    outs_pool = ctx.enter_context(tc.tile_pool(name="outs", bufs=4))

    ps = ctx.enter_context(tc.tile_pool(name="ps", bufs=2, space="PSUM"))

    # ---- constants into SBUF ----
    wkq_sb = consts.tile([C, G], F32)
    nc.sync.dma_start(out=wkq_sb, in_=wkq_d)
    wv_sb = consts.tile([C, CS], F32)
    nc.sync.dma_start(out=wv_sb, in_=wv_d)
    denk_sb = consts.tile([G, 9], F32)
    nc.sync.dma_start(out=denk_sb, in_=denk_d)
    kmat_sb = consts.tile([G, 9, 2, 128], BF16)
    nc.gpsimd.dma_start(out=kmat_sb, in_=kmat_d.transpose([2, 0, 1, 3]))
    woutT_sb = consts.tile([128, 2, CS], BF16)
    nc.gpsimd.dma_start(out=woutT_sb,
                        in_=woutT_d.rearrange("(kc k) m -> k kc m", kc=2))
    ident_sb = consts.tile([128, 128], BF16)
    nc.gpsimd.dma_start(out=ident_sb, in_=ident_d)

    # ---- persistent padded planes (zero borders set once) ----
    cost_pl = [planes.tile([G, PH, PW], BF16, tag=f"cost{b}", name=f"cost_pl{b}") for b in range(BPC)]
    v_pl = [[planes.tile([128, PH, PW], BF16, tag=f"v{b}_{chn}", name=f"v_pl{b}_{chn}") for chn in range(2)]
            for b in range(BPC)]
    for pl in [cost_pl[b] for b in range(BPC)] + [v_pl[b][c] for b in range(BPC)
                                                 for c in range(2)]:
        # interior is fully overwritten every batch; only borders must be 0
        nc.gpsimd.memset(pl[:, 0, :], 0.0)
        nc.gpsimd.memset(pl[:, PH - 1, :], 0.0)
        nc.gpsimd.memset(pl[:, 1:PH - 1, 0], 0.0)
        nc.gpsimd.memset(pl[:, 1:PH - 1, PW - 1], 0.0)

    n_row_tiles = 7          # 56 rows in tiles of 8 -> matmul N=448
    RT = H // n_row_tiles    # 8 rows per tile

    def cview(b, di, dj):
        return cost_pl[b][:, 1 + di:1 + di + 2 * HO:2, 1 + dj:1 + dj + 2 * WO:2]

    # ---- phase A: load x, qkT + exp, v matmuls (both batches) ----
    x_sb = {}
    for b in range(BPC):
        x_sb[b] = xpool.tile([C, H, W], F32, name=f"x_sb{b}")
        nc.sync.dma_start(out=x_sb[b], in_=x_d[b])
    for b in range(BPC):
        for rt in range(n_row_tiles):
            qk_ps = ps.tile([G, RT, W], F32, tag="mm", bufs=2, name="qk_ps")
            nc.tensor.matmul(qk_ps, wkq_sb, x_sb[b][:, rt * RT:(rt + 1) * RT, :],
                             start=True, stop=True)
            nc.scalar.activation(
                out=cost_pl[b][:, 1 + rt * RT:1 + (rt + 1) * RT, 1:1 + W],
                in_=qk_ps, func=mybir.ActivationFunctionType.Exp)
        for chn in range(2):
            for rt in range(n_row_tiles):
                v_ps = ps.tile([128, RT, W], F32, tag="mm", bufs=2, name="v_ps")
                nc.tensor.matmul(v_ps, wv_sb[:, chn * 128:(chn + 1) * 128],
                                 x_sb[b][:, rt * RT:(rt + 1) * RT, :],
                                 start=True, stop=True)
                nc.scalar.copy(
                    out=v_pl[b][chn][:, 1 + rt * RT:1 + (rt + 1) * RT, 1:1 + W],
                    in_=v_ps)

    # ---- phase B: den conv (3 parallel chains) + r = exp(-ln(den)) ----
    r_sb = {}
    for b in range(BPC):
        chains = []
        for c3 in range(3):
            dc = small.tile([G, HO, WO], F32, tag=f"den{c3}", name=f"den{c3}")
            for k in range(3):
                t = c3 * 3 + k
                di, dj = TAPS[t]
                if k == 0:
                    nc.vector.tensor_scalar_mul(dc, cview(b, di, dj),
                                                denk_sb[:, t:t + 1])
                else:
                    nc.vector.scalar_tensor_tensor(
                        out=dc, in0=cview(b, di, dj), scalar=denk_sb[:, t:t + 1],
                        in1=dc, op0=mybir.AluOpType.mult, op1=mybir.AluOpType.add)
            chains.append(dc)
        den = small.tile([G, HO, WO], F32, tag="dent", name="dent")
        nc.vector.tensor_add(den, chains[0], chains[1])
        nc.vector.tensor_add(den, den, chains[2])
        lden = small.tile([G, HO, WO], F32, tag="lden", name="lden")
        nc.scalar.activation(out=lden, in_=den,
                             func=mybir.ActivationFunctionType.Ln)
        r_sb[b] = small.tile([G, HO, WO], F32, tag="rr", name="rr")
        nc.scalar.activation(out=r_sb[b], in_=lden, scale=-1.0,
                             func=mybir.ActivationFunctionType.Exp)

    # ---- phase C: rc[t] = cost_t * r ----
    rc_st = {}
    for b in range(BPC):
        rc_st[b] = [rcpool.tile([G, HO, WO], BF16, tag=f"rc{t}",
                                name=f"rc_st{t}") for t in range(9)]
        for t, (di, dj) in enumerate(TAPS):
            nc.vector.tensor_mul(rc_st[b][t], cview(b, di, dj), r_sb[b])

    # ---- phase D: gamma matmuls, fused product, identity-accum, opre ----
    opre_sb = {}
    for b in range(BPC):
        for chn in range(2):
            accs = [ps.tile([128, ROWS_PER_STRIP, WO], F32, tag="acc", bufs=2,
                            name=f"acc_ps{s}") for s in range(N_STRIPS)]
            for t, (di, dj) in enumerate(TAPS):
                # one 2-bank psum tile holds both strips of gamma_bc
                gam_ps = ps.tile([128, 2, 512], F32, tag="gam", bufs=2,
                                 name="gam_ps")
                for s in range(N_STRIPS):
                    r0 = s * ROWS_PER_STRIP
                    gv = gam_ps[:, s, :ROWS_PER_STRIP * WO].rearrange(
                        "p (a c) -> p a c", a=ROWS_PER_STRIP)
                    nc.tensor.matmul(gv, kmat_sb[:, t, chn, :],
                                     rc_st[b][t][:, r0:r0 + ROWS_PER_STRIP, :],
                                     start=True, stop=True)
                # one full-size product op (reads gamma straight from psum)
                p_sb = prod_pool.tile([128, 2, ROWS_PER_STRIP, WO], BF16)
                gfull = gam_ps[:, :, :ROWS_PER_STRIP * WO].rearrange(
                    "p s (a c) -> p s a c", a=ROWS_PER_STRIP)
                vv = v_pl[b][chn][:, 1 + di:1 + di + 2 * HO:2,
                                  1 + dj:1 + dj + 2 * WO:2].rearrange(
                    "p (s a) c -> p s a c", s=2)
                nc.vector.tensor_mul(p_sb, gfull, vv)
                for s in range(N_STRIPS):
                    nc.tensor.matmul(accs[s], ident_sb, p_sb[:, s],
                                     start=(t == 0), stop=(t == 8))
            for s in range(N_STRIPS):
                o_sb = opre_pool.tile([128, ROWS_PER_STRIP, WO], BF16,
                                      tag=f"opre{chn}_{s}", name=f"opre{chn}{s}")
                nc.scalar.copy(out=o_sb, in_=accs[s])
                opre_sb[(b, chn, s)] = o_sb

    # ---- phase E: Wout projection + store ----
    for b in range(BPC):
        for mo in range(2):
            for s in range(N_STRIPS):
                out_ps = ps.tile([128, ROWS_PER_STRIP, WO], F32, tag="mm",
                                 bufs=2, name="out_ps")
                for kc in range(2):
                    nc.tensor.matmul(out_ps,
                                     woutT_sb[:, kc, mo * 128:(mo + 1) * 128],
                                     opre_sb[(b, kc, s)],
                                     start=(kc == 0), stop=(kc == 1))
                o_final = outs_pool.tile([128, ROWS_PER_STRIP, WO], F32)
                nc.scalar.copy(out=o_final, in_=out_ps)
                nc.sync.dma_start(
                    out=out_d[b, mo * 128:(mo + 1) * 128,
                              s * ROWS_PER_STRIP:(s + 1) * ROWS_PER_STRIP, :],
                    in_=o_final)


def _install_ntff_shim():
    """bass_utils expects antenv.axon_hooks (absent in this checkout); shim it
    with the ctypes NTFF hook from trn_agent_boot so trace=True works."""
    import sys
    import types
    try:
        from antenv.axon_hooks import get_axon_ntff_profile_hook  # noqa: F401
        return
    except ImportError:
        pass
    try:
        from trn_agent_boot.trn_boot import _ntff_profile_via_ctypes
        hook = _ntff_profile_via_ctypes("/opt/axon/libaxon_pjrt.so")
    except Exception:
        hook = None
    mod = types.ModuleType("antenv.axon_hooks")
    mod._hook = hook
    mod.get_axon_ntff_profile_hook = lambda: mod._hook
    mod.set_axon_ntff_profile_hook = lambda h: setattr(mod, "_hook", h)
    sys.modules["antenv.axon_hooks"] = mod


def _get_program():
    if "nc" not in _BUILD_CACHE:
        _BUILD_CACHE["nc"] = _build_program()
    return _BUILD_CACHE["nc"]


def kernel(x, Wk, Wv, Wout, q_param, attn_scale, rpb_table):
    x = np.ascontiguousarray(np.asarray(x, dtype=np.float32))
    wts = _host_weights(np.asarray(Wk), np.asarray(Wv), np.asarray(Wout),
                        np.asarray(q_param), np.asarray(attn_scale),
                        np.asarray(rpb_table))
    nc = _get_program()

    in_maps = []
    for c in range(NCORES):
        in_maps.append({
            "x": np.ascontiguousarray(x[c * BPC:(c + 1) * BPC]),
            "wkq": wts["wkq"], "wv": wts["wv"], "denk": wts["denk"],
            "kmat": wts["kmat"], "woutT": wts["woutT"], "ident": wts["ident"],
        })

    trace = bool(int(os.environ.get("KERNEL_TRACE", "0")))
    if trace:
        _install_ntff_shim()
    res = run_bass_kernel_spmd(nc, in_maps, core_ids=list(range(NCORES)),
                               trace=trace)
    _BUILD_CACHE["last_results"] = res

    out = np.empty((B, CS, HO, WO), np.float32)
    for c in range(NCORES):
        out[c * BPC:(c + 1) * BPC] = res.results[c]["out"]
    return out


# revision 16
# speedup vs baseline: 1.4272x; 1.0210x over previous
"""Trainium2 Bass kernel for nn_FusedKQnA (sparse attention with learned
queries + depthwise stride-2 conv aggregation).

Math restructuring (vs the reference):
  - k is never materialized: qkT = x^T @ (Wk @ QW) with QW the block-diagonal
    arrangement of the scaled learned queries -> one (128->32) matmul.
  - The global max subtractions inside the two exp() calls cancel exactly
    between numerator and denominator, so they are dropped.
  - The 1024-channel depthwise conv never materializes.  With
    r = 1/sum_den (computed as exp(-ln(den)), same ACT table set) define
        gamma[t,h,ij] = sum_q kern[t,q*8+h] * r[q*8+h,ij] * cost[n_t(ij),q*8+h]
    Then out_pre[(h,c),ij] = sum_t gamma[t,h,ij] * v[n_t(ij),(h,c)]  (256 ch)
    and out = Wout @ out_pre.
  - gamma's q-contraction + broadcast over the 32 channels of each head is a
    single small PE matmul per (tap, channel-chunk) with a one-hot*kern
    stationary operand; the tap accumulation is PSUM accumulation through
    identity matmuls.

Sharding: pure data parallel over batch: 16 batches -> 8 cores x 2.
"""

import os
from contextlib import ExitStack

import numpy as np

import concourse.bass as bass
import concourse.mybir as mybir
import concourse.tile as tile
from concourse import bacc
from concourse.bass_utils import run_bass_kernel_spmd

# Problem constants (hardcoded per spec nn_FusedKQnA_1726576854813)
N_Q, N_HEADS, KSIZE, STRIDE, PADDING = 4, 4, 3, 2, 1
B, C, H, W = 16, 128, 56, 56
HC = C // N_HEADS            # 32 head channels
HP = N_HEADS * STRIDE        # 8 effective heads
CS = C * STRIDE              # 256
G = N_Q * HP                 # 32 kernel groups
HO, WO = H // STRIDE, W // STRIDE   # 28, 28
NCORES = 8
BPC = B // NCORES            # batches per core

TAPS = [(di, dj) for di in (-1, 0, 1) for dj in (-1, 0, 1)]
N_STRIPS = 2                 # output rows split into strips of 14 (392 px)
ROWS_PER_STRIP = HO // N_STRIPS

F32 = mybir.dt.float32
BF16 = mybir.dt.bfloat16

_BUILD_CACHE = {}


def _host_weights(Wk, Wv, Wout, q_param, attn_scale, rpb_table):
    """Precompute all small weight tensors on the host."""
    q = q_param.reshape(N_Q, HP, HC).astype(np.float64) * (HC ** -0.5)
    QW = np.zeros((CS, G), np.float64)
    for qi in range(N_Q):
        for h in range(HP):
            QW[h * HC:(h + 1) * HC, qi * HP + h] = q[qi, h]
    wkq = (Wk.astype(np.float64) @ QW).astype(np.float32)        # (128, 32)

    rpb_exp = np.exp(rpb_table.astype(np.float64))               # (9, 32)
    kern_num = (rpb_exp * attn_scale.astype(np.float64))         # (9, 32)

    # denominator conv kernels as diagonal matmul weights: (9, G, G)
    denk = np.zeros((KSIZE * KSIZE, G, G), np.float32)
    for t in range(KSIZE * KSIZE):
        np.fill_diagonal(denk[t], rpb_exp[t])

    # gamma-broadcast stationary operands: kmat[t, ch] has shape (32, 128)
    # kmat[t,ch][g, m] = kern_num[t, g] if g % HP == ch*4 + m//HC else 0
    kmat = np.zeros((KSIZE * KSIZE, 2, G, 128), np.float32)
    for t in range(KSIZE * KSIZE):
        for ch in range(2):
            for g in range(G):
                h = g % HP
                if h // 4 == ch:
                    m0 = (h % 4) * HC
                    kmat[t, ch, g, m0:m0 + HC] = kern_num[t, g]

    woutT = np.ascontiguousarray(Wout.T.astype(np.float32))      # (256, 256) lhsT
    ident = np.eye(128, dtype=np.float32)
    return dict(wkq=wkq, denk=denk, kmat=kmat, woutT=woutT, ident=ident,
                wv=np.ascontiguousarray(Wv.astype(np.float32)))


def _build_program():
    """Build the Bass/Tile program once. Returns (nc, input_names)."""
    nc = bacc.Bacc("TRN2", target_bir_lowering=False, debug=False,
                   enable_asserts=False, num_devices=NCORES)

    x_d = nc.dram_tensor("x", [BPC, C, H, W], F32, kind="ExternalInput").ap()
    wkq_d = nc.dram_tensor("wkq", [C, G], F32, kind="ExternalInput").ap()
    wv_d = nc.dram_tensor("wv", [C, CS], F32, kind="ExternalInput").ap()
    denk_d = nc.dram_tensor("denk", [9, G, G], F32, kind="ExternalInput").ap()
    kmat_d = nc.dram_tensor("kmat", [9, 2, G, 128], F32, kind="ExternalInput").ap()
    woutT_d = nc.dram_tensor("woutT", [CS, CS], F32, kind="ExternalInput").ap()
    ident_d = nc.dram_tensor("ident", [128, 128], F32, kind="ExternalInput").ap()
    out_d = nc.dram_tensor("out", [BPC, CS, HO, WO], F32, kind="ExternalOutput").ap()

    with tile.TileContext(nc) as tc, ExitStack() as ctx:
        _kernel_body(ctx, tc, out_d, x_d, wkq_d, wv_d, denk_d, kmat_d,
                     woutT_d, ident_d)

    _pin_act_tables()
    nc.compile()
    return nc


def _pin_act_tables():
    """Force one ACT table set (natural_log_exp_and_others) for Exp+Ln so the
    scheduler doesn't thrash table loads between them."""
    from concourse import hw_specs
    import concourse.bacc as bacc_mod
    if getattr(bacc_mod, "_act_tables_pinned", False):
        return
    orig = hw_specs.get_activation_tables

    def patched(arch):
        tabs = dict(orig(arch))
        keep = "natural_log_exp_and_others"
        for name in list(tabs):
            if name == keep:
                continue
            fns = tabs[name]
            if any(str(f).endswith((".Exp", ".Ln")) for f in fns):
                tabs[name] = type(fns)()
        return tabs

    bacc_mod.get_activation_tables = patched
    bacc_mod._act_tables_pinned = True


def _kernel_body(ctx, tc, out_d, x_d, wkq_d, wv_d, denk_d, kmat_d,
                 woutT_d, ident_d):
    nc = tc.nc
    PH, PW = H + 2, W + 2        # padded plane 58 x 58

    consts = ctx.enter_context(tc.tile_pool(name="consts", bufs=1))
    planes = ctx.enter_context(tc.tile_pool(name="planes", bufs=1))
    xpool = ctx.enter_context(tc.tile_pool(name="xpool", bufs=2))
    small = ctx.enter_context(tc.tile_pool(name="small", bufs=2))
    rcpool = ctx.enter_context(tc.tile_pool(name="rcpool", bufs=2))
    prod_pool = ctx.enter_context(tc.tile_pool(name="prod", bufs=4))
    opre_pool = ctx.enter_context(tc.tile_pool(name="opre", bufs=2))
    outs_pool = ctx.enter_context(tc.tile_pool(name="outs", bufs=4))

    ps = ctx.enter_context(tc.tile_pool(name="ps", bufs=2, space="PSUM"))

    # ---- constants into SBUF ----
    wkq_sb = consts.tile([C, G], F32)
    nc.sync.dma_start(out=wkq_sb, in_=wkq_d)
    wv_sb = consts.tile([C, CS], F32)
    nc.sync.dma_start(out=wv_sb, in_=wv_d)
    denk_sb = consts.tile([G, 9, G], BF16)
    nc.gpsimd.dma_start(out=denk_sb, in_=denk_d.transpose([1, 0, 2]))
    kmat_sb = consts.tile([G, 9, 2, 128], BF16)
    nc.gpsimd.dma_start(out=kmat_sb, in_=kmat_d.transpose([2, 0, 1, 3]))
    woutT_sb = consts.tile([128, 2, CS], BF16)
    nc.gpsimd.dma_start(out=woutT_sb,
                        in_=woutT_d.rearrange("(kc k) m -> k kc m", kc=2))
    ident_sb = consts.tile([128, 128], BF16)
    nc.gpsimd.dma_start(out=ident_sb, in_=ident_d)

    # ---- persistent padded planes (zero borders set once) ----
    cost_pl = [planes.tile([G, PH, PW], BF16, tag=f"cost{b}", name=f"cost_pl{b}") for b in range(BPC)]
    v_pl = [[planes.tile([128, PH, PW], BF16, tag=f"v{b}_{chn}", name=f"v_pl{b}_{chn}") for chn in range(2)]
            for b in range(BPC)]
    for pl in [cost_pl[b] for b in range(BPC)] + [v_pl[b][c] for b in range(BPC)
                                                 for c in range(2)]:
        # interior is fully overwritten every batch; only borders must be 0
        nc.gpsimd.memset(pl[:, 0, :], 0.0)
        nc.gpsimd.memset(pl[:, PH - 1, :], 0.0)
        nc.gpsimd.memset(pl[:, 1:PH - 1, 0], 0.0)
        nc.gpsimd.memset(pl[:, 1:PH - 1, PW - 1], 0.0)

    n_row_tiles = 7          # 56 rows in tiles of 8 -> matmul N=448
    RT = H // n_row_tiles    # 8 rows per tile

    def cview(b, di, dj):
        return cost_pl[b][:, 1 + di:1 + di + 2 * HO:2, 1 + dj:1 + dj + 2 * WO:2]

    # ---- phase A: load x, qkT + exp, v matmuls (both batches) ----
    x_sb = {}
    for b in range(BPC):
        x_sb[b] = xpool.tile([C, H, W], F32, name=f"x_sb{b}")
        nc.sync.dma_start(out=x_sb[b], in_=x_d[b])
    for b in range(BPC):
        for rt in range(n_row_tiles):
            qk_ps = ps.tile([G, RT, W], F32, tag="mm", bufs=2, name="qk_ps")
            nc.tensor.matmul(qk_ps, wkq_sb, x_sb[b][:, rt * RT:(rt + 1) * RT, :],
                             start=True, stop=True)
            nc.scalar.activation(
                out=cost_pl[b][:, 1 + rt * RT:1 + (rt + 1) * RT, 1:1 + W],
                in_=qk_ps, func=mybir.ActivationFunctionType.Exp)
        for chn in range(2):
            for rt in range(n_row_tiles):
                v_ps = ps.tile([128, RT, W], F32, tag="mm", bufs=2, name="v_ps")
                nc.tensor.matmul(v_ps, wv_sb[:, chn * 128:(chn + 1) * 128],
                                 x_sb[b][:, rt * RT:(rt + 1) * RT, :],
                                 start=True, stop=True)
                if chn == 0:
                    nc.scalar.copy(
                        out=v_pl[b][chn][:, 1 + rt * RT:1 + (rt + 1) * RT,
                                         1:1 + W],
                        in_=v_ps)
                else:
                    nc.vector.tensor_copy(
                        out=v_pl[b][chn][:, 1 + rt * RT:1 + (rt + 1) * RT,
                                         1:1 + W],
                        in_=v_ps)

    # ---- phase B: den conv as diagonal matmuls on PE + r = exp(-ln(den)) ----
    r_sb = {}
    for b in range(BPC):
        den_ps = ps.tile([G, 2, 512], F32, tag="gam", bufs=2, name="den_ps")
        for s in range(N_STRIPS):
            r0 = s * ROWS_PER_STRIP
            dv = den_ps[:, s, :ROWS_PER_STRIP * WO].rearrange(
                "p (a c) -> p a c", a=ROWS_PER_STRIP)
            for t, (di, dj) in enumerate(TAPS):
                nc.tensor.matmul(
                    dv, denk_sb[:, t, :],
                    cview(b, di, dj)[:, r0:r0 + ROWS_PER_STRIP, :],
                    start=(t == 0), stop=(t == 8))
        lden = small.tile([G, 2, ROWS_PER_STRIP, WO], F32, tag="lden",
                          name="lden")
        dfull = den_ps[:, :, :ROWS_PER_STRIP * WO].rearrange(
            "p s (a c) -> p s a c", a=ROWS_PER_STRIP)
        nc.scalar.activation(out=lden, in_=dfull,
                             func=mybir.ActivationFunctionType.Ln)
        r_sb[b] = small.tile([G, 2, ROWS_PER_STRIP, WO], BF16, tag="rr",
                             name="rr")
        nc.scalar.activation(out=r_sb[b], in_=lden, scale=-1.0,
                             func=mybir.ActivationFunctionType.Exp)

    # ---- phase C: rc[t] = cost_t * r (on GpSimd; frees Vector) ----
    rc_st = {}
    for b in range(BPC):
        rc_st[b] = [rcpool.tile([G, 2, ROWS_PER_STRIP, WO], BF16, tag=f"rc{t}",
                                name=f"rc_st{t}") for t in range(9)]
        for t, (di, dj) in enumerate(TAPS):
            cvs = cview(b, di, dj).rearrange("p (s a) c -> p s a c", s=2)
            nc.gpsimd.tensor_mul(rc_st[b][t], cvs, r_sb[b])

    # ---- phases D+E per batch: gamma/product/accum then Wout ----
    opre_sb = {}
    for b in range(BPC):
        for chn in range(2):
            accs = [ps.tile([128, ROWS_PER_STRIP, WO], F32, tag="acc", bufs=2,
                            name=f"acc_ps{s}") for s in range(N_STRIPS)]
            for t, (di, dj) in enumerate(TAPS):
                # one 2-bank psum tile holds both strips of gamma_bc
                gam_ps = ps.tile([128, 2, 512], F32, tag="gam", bufs=2,
                                 name="gam_ps")
                for s in range(N_STRIPS):
                    gv = gam_ps[:, s, :ROWS_PER_STRIP * WO].rearrange(
                        "p (a c) -> p a c", a=ROWS_PER_STRIP)
                    nc.tensor.matmul(gv, kmat_sb[:, t, chn, :],
                                     rc_st[b][t][:, s],
                                     start=True, stop=True)
                # one full-size product op (reads gamma straight from psum)
                p_sb = prod_pool.tile([128, 2, ROWS_PER_STRIP, WO], BF16)
                gfull = gam_ps[:, :, :ROWS_PER_STRIP * WO].rearrange(
                    "p s (a c) -> p s a c", a=ROWS_PER_STRIP)
                vv = v_pl[b][chn][:, 1 + di:1 + di + 2 * HO:2,
                                  1 + dj:1 + dj + 2 * WO:2].rearrange(
                    "p (s a) c -> p s a c", s=2)
                nc.vector.tensor_mul(p_sb, gfull, vv)
                for s in range(N_STRIPS):
                    nc.tensor.matmul(accs[s], ident_sb, p_sb[:, s],
                                     start=(t == 0), stop=(t == 8))
            for s in range(N_STRIPS):
                o_sb = opre_pool.tile([128, ROWS_PER_STRIP, WO], BF16,
                                      tag=f"opre{chn}_{s}", name=f"opre{chn}{s}")
                nc.scalar.copy(out=o_sb, in_=accs[s])
                opre_sb[(b, chn, s)] = o_sb

        for mo in range(2):
            for s in range(N_STRIPS):
                out_ps = ps.tile([128, ROWS_PER_STRIP, WO], F32, tag="mm",
                                 bufs=2, name="out_ps")
                for kc in range(2):
                    nc.tensor.matmul(out_ps,
                                     woutT_sb[:, kc, mo * 128:(mo + 1) * 128],
                                     opre_sb[(b, kc, s)],
                                     start=(kc == 0), stop=(kc == 1))
                o_final = outs_pool.tile([128, ROWS_PER_STRIP, WO], F32)
                nc.scalar.copy(out=o_final, in_=out_ps)
                nc.sync.dma_start(
                    out=out_d[b, mo * 128:(mo + 1) * 128,
                              s * ROWS_PER_STRIP:(s + 1) * ROWS_PER_STRIP, :],
                    in_=o_final)


def _install_ntff_shim():
    """bass_utils expects antenv.axon_hooks (absent in this checkout); shim it
    with the ctypes NTFF hook from trn_agent_boot so trace=True works."""
    import sys
    import types
    try:
        from antenv.axon_hooks import get_axon_ntff_profile_hook  # noqa: F401
        return
    except ImportError:
        pass
    try:
        from trn_agent_boot.trn_boot import _ntff_profile_via_ctypes
        hook = _ntff_profile_via_ctypes("/opt/axon/libaxon_pjrt.so")
    except Exception:
        hook = None
    mod = types.ModuleType("antenv.axon_hooks")
    mod._hook = hook
    mod.get_axon_ntff_profile_hook = lambda: mod._hook
    mod.set_axon_ntff_profile_hook = lambda h: setattr(mod, "_hook", h)
    sys.modules["antenv.axon_hooks"] = mod


def _get_program():
    if "nc" not in _BUILD_CACHE:
        _BUILD_CACHE["nc"] = _build_program()
    return _BUILD_CACHE["nc"]


def kernel(x, Wk, Wv, Wout, q_param, attn_scale, rpb_table):
    x = np.ascontiguousarray(np.asarray(x, dtype=np.float32))
    wts = _host_weights(np.asarray(Wk), np.asarray(Wv), np.asarray(Wout),
                        np.asarray(q_param), np.asarray(attn_scale),
                        np.asarray(rpb_table))
    nc = _get_program()

    in_maps = []
    for c in range(NCORES):
        in_maps.append({
            "x": np.ascontiguousarray(x[c * BPC:(c + 1) * BPC]),
            "wkq": wts["wkq"], "wv": wts["wv"], "denk": wts["denk"],
            "kmat": wts["kmat"], "woutT": wts["woutT"], "ident": wts["ident"],
        })

    trace = bool(int(os.environ.get("KERNEL_TRACE", "0")))
    if trace:
        _install_ntff_shim()
    res = run_bass_kernel_spmd(nc, in_maps, core_ids=list(range(NCORES)),
                               trace=trace)
    _BUILD_CACHE["last_results"] = res

    out = np.empty((B, CS, HO, WO), np.float32)
    for c in range(NCORES):
        out[c * BPC:(c + 1) * BPC] = res.results[c]["out"]
    return out


# revision 18
# speedup vs baseline: 1.5843x; 1.1101x over previous
"""Trainium2 Bass kernel for nn_FusedKQnA (sparse attention with learned
queries + depthwise stride-2 conv aggregation).

Math restructuring (vs the reference):
  - k is never materialized: qkT = x^T @ (Wk @ QW) with QW the block-diagonal
    arrangement of the scaled learned queries -> one (128->32) matmul.
  - The global max subtractions inside the two exp() calls cancel exactly
    between numerator and denominator, so they are dropped.
  - The 1024-channel depthwise conv never materializes.  With
    r = 1/sum_den (computed as exp(-ln(den)), same ACT table set) define
        gamma[t,h,ij] = sum_q kern[t,q*8+h] * r[q*8+h,ij] * cost[n_t(ij),q*8+h]
    Then out_pre[(h,c),ij] = sum_t gamma[t,h,ij] * v[n_t(ij),(h,c)]  (256 ch)
    and out = Wout @ out_pre.
  - gamma's q-contraction + broadcast over the 32 channels of each head is a
    single small PE matmul per (tap, channel-chunk) with a one-hot*kern
    stationary operand; the tap accumulation is PSUM accumulation through
    identity matmuls.

Sharding: pure data parallel over batch: 16 batches -> 8 cores x 2.
"""

import os
from contextlib import ExitStack

import numpy as np

import concourse.bass as bass
import concourse.mybir as mybir
import concourse.tile as tile
from concourse import bacc
from concourse.bass_utils import run_bass_kernel_spmd

# Problem constants (hardcoded per spec nn_FusedKQnA_1726576854813)
N_Q, N_HEADS, KSIZE, STRIDE, PADDING = 4, 4, 3, 2, 1
B, C, H, W = 16, 128, 56, 56
HC = C // N_HEADS            # 32 head channels
HP = N_HEADS * STRIDE        # 8 effective heads
CS = C * STRIDE              # 256
G = N_Q * HP                 # 32 kernel groups
HO, WO = H // STRIDE, W // STRIDE   # 28, 28
NCORES = 8
BPC = B // NCORES            # batches per core

TAPS = [(di, dj) for di in (-1, 0, 1) for dj in (-1, 0, 1)]
N_STRIPS = 2                 # output rows split into strips of 14 (392 px)
ROWS_PER_STRIP = HO // N_STRIPS

F32 = mybir.dt.float32
BF16 = mybir.dt.bfloat16

_BUILD_CACHE = {}


def _host_weights(Wk, Wv, Wout, q_param, attn_scale, rpb_table):
    """Precompute all small weight tensors on the host."""
    q = q_param.reshape(N_Q, HP, HC).astype(np.float64) * (HC ** -0.5)
    QW = np.zeros((CS, G), np.float64)
    for qi in range(N_Q):
        for h in range(HP):
            QW[h * HC:(h + 1) * HC, qi * HP + h] = q[qi, h]
    wkq = (Wk.astype(np.float64) @ QW).astype(np.float32)        # (128, 32)

    rpb_exp = np.exp(rpb_table.astype(np.float64))               # (9, 32)
    kern_num = (rpb_exp * attn_scale.astype(np.float64))         # (9, 32)

    # denominator conv kernels as diagonal matmul weights: (9, G, G)
    denk = np.zeros((KSIZE * KSIZE, G, G), np.float32)
    for t in range(KSIZE * KSIZE):
        np.fill_diagonal(denk[t], rpb_exp[t])

    # gamma-broadcast stationary operands: kmat[t, ch] has shape (32, 128)
    # kmat[t,ch][g, m] = kern_num[t, g] if g % HP == ch*4 + m//HC else 0
    kmat = np.zeros((KSIZE * KSIZE, 2, G, 128), np.float32)
    for t in range(KSIZE * KSIZE):
        for ch in range(2):
            for g in range(G):
                h = g % HP
                if h // 4 == ch:
                    m0 = (h % 4) * HC
                    kmat[t, ch, g, m0:m0 + HC] = kern_num[t, g]

    woutT = np.ascontiguousarray(Wout.T.astype(np.float32))      # (256, 256) lhsT
    ident = np.eye(128, dtype=np.float32)
    import ml_dtypes
    return dict(wkq=wkq.astype(ml_dtypes.bfloat16), denk=denk, kmat=kmat,
                woutT=woutT, ident=ident,
                wv=np.ascontiguousarray(Wv.astype(ml_dtypes.bfloat16)))


def _build_program():
    """Build the Bass/Tile program once. Returns (nc, input_names)."""
    nc = bacc.Bacc("TRN2", target_bir_lowering=False, debug=False,
                   enable_asserts=False, num_devices=NCORES)

    x_d = nc.dram_tensor("x", [BPC, C, H, W], BF16, kind="ExternalInput").ap()
    wkq_d = nc.dram_tensor("wkq", [C, G], BF16, kind="ExternalInput").ap()
    wv_d = nc.dram_tensor("wv", [C, CS], BF16, kind="ExternalInput").ap()
    denk_d = nc.dram_tensor("denk", [9, G, G], F32, kind="ExternalInput").ap()
    kmat_d = nc.dram_tensor("kmat", [9, 2, G, 128], F32, kind="ExternalInput").ap()
    woutT_d = nc.dram_tensor("woutT", [CS, CS], F32, kind="ExternalInput").ap()
    ident_d = nc.dram_tensor("ident", [128, 128], F32, kind="ExternalInput").ap()
    out_d = nc.dram_tensor("out", [BPC, CS, HO, WO], F32, kind="ExternalOutput").ap()

    with tile.TileContext(nc) as tc, ExitStack() as ctx:
        _kernel_body(ctx, tc, out_d, x_d, wkq_d, wv_d, denk_d, kmat_d,
                     woutT_d, ident_d)

    _pin_act_tables()
    nc.compile()
    return nc


def _pin_act_tables():
    """Force one ACT table set (natural_log_exp_and_others) for Exp+Ln so the
    scheduler doesn't thrash table loads between them."""
    from concourse import hw_specs
    import concourse.bacc as bacc_mod
    if getattr(bacc_mod, "_act_tables_pinned", False):
        return
    orig = hw_specs.get_activation_tables

    def patched(arch):
        tabs = dict(orig(arch))
        keep = "natural_log_exp_and_others"
        for name in list(tabs):
            if name == keep:
                continue
            fns = tabs[name]
            if any(str(f).endswith((".Exp", ".Ln")) for f in fns):
                tabs[name] = type(fns)()
        return tabs

    bacc_mod.get_activation_tables = patched
    bacc_mod._act_tables_pinned = True


def _kernel_body(ctx, tc, out_d, x_d, wkq_d, wv_d, denk_d, kmat_d,
                 woutT_d, ident_d):
    nc = tc.nc
    PH, PW = H + 2, W + 2        # padded plane 58 x 58

    consts = ctx.enter_context(tc.tile_pool(name="consts", bufs=1))
    planes = ctx.enter_context(tc.tile_pool(name="planes", bufs=1))
    xpool = ctx.enter_context(tc.tile_pool(name="xpool", bufs=2))
    small = ctx.enter_context(tc.tile_pool(name="small", bufs=2))
    rcpool = ctx.enter_context(tc.tile_pool(name="rcpool", bufs=2))
    prod_pool = ctx.enter_context(tc.tile_pool(name="prod", bufs=4))
    opre_pool = ctx.enter_context(tc.tile_pool(name="opre", bufs=2))
    outs_pool = ctx.enter_context(tc.tile_pool(name="outs", bufs=4))

    ps = ctx.enter_context(tc.tile_pool(name="ps", bufs=2, space="PSUM"))

    # ---- constants into SBUF ----
    wkq_sb = consts.tile([C, G], BF16)
    nc.sync.dma_start(out=wkq_sb, in_=wkq_d)
    wv_sb = consts.tile([C, CS], BF16)
    nc.sync.dma_start(out=wv_sb, in_=wv_d)
    denk_sb = consts.tile([G, 9, G], BF16)
    nc.gpsimd.dma_start(out=denk_sb, in_=denk_d.transpose([1, 0, 2]))
    kmat_sb = consts.tile([G, 9, 2, 128], BF16)
    nc.gpsimd.dma_start(out=kmat_sb, in_=kmat_d.transpose([2, 0, 1, 3]))
    woutT_sb = consts.tile([128, 2, CS], BF16)
    nc.gpsimd.dma_start(out=woutT_sb,
                        in_=woutT_d.rearrange("(kc k) m -> k kc m", kc=2))
    ident_sb = consts.tile([128, 128], BF16)
    nc.gpsimd.dma_start(out=ident_sb, in_=ident_d)

    # ---- PE warm-up: ~3.5us of back-to-back matmuls during the x DMA so
    #      the HAM clock-gate opens (1.2 -> 2.4 GHz) before real work ----
    warm_sb = consts.tile([128, 128], BF16, name="warm_sb")
    nc.gpsimd.memset(warm_sb, 0.0)
    warm_ps = ps.tile([128, 128], F32, tag="gam", bufs=2, name="warm_ps")
    for _ in range(40):
        nc.tensor.matmul(warm_ps, warm_sb, warm_sb, start=True, stop=True)

    # ---- persistent padded planes (zero borders set once) ----
    cost_pl = [planes.tile([G, PH, PW], BF16, tag=f"cost{b}", name=f"cost_pl{b}") for b in range(BPC)]
    v_pl = [[planes.tile([128, PH, PW], BF16, tag=f"v{b}_{chn}", name=f"v_pl{b}_{chn}") for chn in range(2)]
            for b in range(BPC)]
    for pl in [cost_pl[b] for b in range(BPC)] + [v_pl[b][c] for b in range(BPC)
                                                 for c in range(2)]:
        # interior is fully overwritten every batch; only borders must be 0
        nc.gpsimd.memset(pl[:, 0, :], 0.0)
        nc.gpsimd.memset(pl[:, PH - 1, :], 0.0)
        nc.gpsimd.memset(pl[:, 1:PH - 1, 0], 0.0)
        nc.gpsimd.memset(pl[:, 1:PH - 1, PW - 1], 0.0)

    n_row_tiles = 7          # 56 rows in tiles of 8 -> matmul N=448
    RT = H // n_row_tiles    # 8 rows per tile

    def cview(b, di, dj):
        return cost_pl[b][:, 1 + di:1 + di + 2 * HO:2, 1 + dj:1 + dj + 2 * WO:2]

    # ---- phase A: load x, qkT + exp, v matmuls (both batches) ----
    x_sb = {}
    for b in range(BPC):
        x_sb[b] = xpool.tile([C, H, W], BF16, name=f"x_sb{b}")
        nc.sync.dma_start(out=x_sb[b], in_=x_d[b])
    for b in range(BPC):
        for rt in range(n_row_tiles):
            qk_ps = ps.tile([G, RT, W], F32, tag="mm", bufs=2, name="qk_ps")
            nc.tensor.matmul(qk_ps, wkq_sb, x_sb[b][:, rt * RT:(rt + 1) * RT, :],
                             start=True, stop=True)
            nc.scalar.activation(
                out=cost_pl[b][:, 1 + rt * RT:1 + (rt + 1) * RT, 1:1 + W],
                in_=qk_ps, func=mybir.ActivationFunctionType.Exp)
        for chn in range(2):
            for rt in range(n_row_tiles):
                v_ps = ps.tile([128, RT, W], F32, tag="mm", bufs=2, name="v_ps")
                nc.tensor.matmul(v_ps, wv_sb[:, chn * 128:(chn + 1) * 128],
                                 x_sb[b][:, rt * RT:(rt + 1) * RT, :],
                                 start=True, stop=True)
                if chn == 0:
                    nc.scalar.copy(
                        out=v_pl[b][chn][:, 1 + rt * RT:1 + (rt + 1) * RT,
                                         1:1 + W],
                        in_=v_ps)
                else:
                    nc.vector.tensor_copy(
                        out=v_pl[b][chn][:, 1 + rt * RT:1 + (rt + 1) * RT,
                                         1:1 + W],
                        in_=v_ps)

    # ---- phase B: den conv as diagonal matmuls on PE + r = exp(-ln(den)) ----
    r_sb = {}
    for b in range(BPC):
        den_ps = ps.tile([G, 2, 512], F32, tag="gam", bufs=2, name="den_ps")
        for s in range(N_STRIPS):
            r0 = s * ROWS_PER_STRIP
            dv = den_ps[:, s, :ROWS_PER_STRIP * WO].rearrange(
                "p (a c) -> p a c", a=ROWS_PER_STRIP)
            for t, (di, dj) in enumerate(TAPS):
                nc.tensor.matmul(
                    dv, denk_sb[:, t, :],
                    cview(b, di, dj)[:, r0:r0 + ROWS_PER_STRIP, :],
                    start=(t == 0), stop=(t == 8))
        lden = small.tile([G, 2, ROWS_PER_STRIP, WO], F32, tag="lden",
                          name="lden")
        dfull = den_ps[:, :, :ROWS_PER_STRIP * WO].rearrange(
            "p s (a c) -> p s a c", a=ROWS_PER_STRIP)
        nc.scalar.activation(out=lden, in_=dfull,
                             func=mybir.ActivationFunctionType.Ln)
        r_sb[b] = small.tile([G, 2, ROWS_PER_STRIP, WO], BF16, tag="rr",
                             name="rr")
        nc.scalar.activation(out=r_sb[b], in_=lden, scale=-1.0,
                             func=mybir.ActivationFunctionType.Exp)

    # ---- phase C: rc[t] = cost_t * r (on GpSimd; frees Vector) ----
    rc_st = {}
    for b in range(BPC):
        rc_st[b] = [rcpool.tile([G, 2, ROWS_PER_STRIP, WO], BF16, tag=f"rc{t}",
                                name=f"rc_st{t}") for t in range(9)]
        for t, (di, dj) in enumerate(TAPS):
            cvs = cview(b, di, dj).rearrange("p (s a) c -> p s a c", s=2)
            nc.gpsimd.tensor_mul(rc_st[b][t], cvs, r_sb[b])

    # ---- phases D+E per batch: gamma/product/accum then Wout ----
    opre_sb = {}
    for b in range(BPC):
        for chn in range(2):
            accs = [ps.tile([128, ROWS_PER_STRIP, WO], F32, tag="acc", bufs=2,
                            name=f"acc_ps{s}") for s in range(N_STRIPS)]
            for t, (di, dj) in enumerate(TAPS):
                # one 2-bank psum tile holds both strips of gamma_bc
                gam_ps = ps.tile([128, 2, 512], F32, tag="gam", bufs=2,
                                 name="gam_ps")
                for s in range(N_STRIPS):
                    gv = gam_ps[:, s, :ROWS_PER_STRIP * WO].rearrange(
                        "p (a c) -> p a c", a=ROWS_PER_STRIP)
                    nc.tensor.matmul(gv, kmat_sb[:, t, chn, :],
                                     rc_st[b][t][:, s],
                                     start=True, stop=True)
                # one full-size product op (reads gamma straight from psum)
                p_sb = prod_pool.tile([128, 2, ROWS_PER_STRIP, WO], BF16)
                gfull = gam_ps[:, :, :ROWS_PER_STRIP * WO].rearrange(
                    "p s (a c) -> p s a c", a=ROWS_PER_STRIP)
                vv = v_pl[b][chn][:, 1 + di:1 + di + 2 * HO:2,
                                  1 + dj:1 + dj + 2 * WO:2].rearrange(
                    "p (s a) c -> p s a c", s=2)
                nc.vector.tensor_mul(p_sb, gfull, vv)
                for s in range(N_STRIPS):
                    nc.tensor.matmul(accs[s], ident_sb, p_sb[:, s],
                                     start=(t == 0), stop=(t == 8))
            for s in range(N_STRIPS):
                o_sb = opre_pool.tile([128, ROWS_PER_STRIP, WO], BF16,
                                      tag=f"opre{chn}_{s}", name=f"opre{chn}{s}")
                nc.scalar.copy(out=o_sb, in_=accs[s])
                opre_sb[(b, chn, s)] = o_sb

        for mo in range(2):
            for s in range(N_STRIPS):
                out_ps = ps.tile([128, ROWS_PER_STRIP, WO], F32, tag="mm",
                                 bufs=2, name="out_ps")
                for kc in range(2):
                    nc.tensor.matmul(out_ps,
                                     woutT_sb[:, kc, mo * 128:(mo + 1) * 128],
                                     opre_sb[(b, kc, s)],
                                     start=(kc == 0), stop=(kc == 1))
                o_final = outs_pool.tile([128, ROWS_PER_STRIP, WO], F32)
                nc.scalar.copy(out=o_final, in_=out_ps)
                nc.sync.dma_start(
                    out=out_d[b, mo * 128:(mo + 1) * 128,
                              s * ROWS_PER_STRIP:(s + 1) * ROWS_PER_STRIP, :],
                    in_=o_final)


def _install_ntff_shim():
    """bass_utils expects antenv.axon_hooks (absent in this checkout); shim it
    with the ctypes NTFF hook from trn_agent_boot so trace=True works."""
    import sys
    import types
    try:
        from antenv.axon_hooks import get_axon_ntff_profile_hook  # noqa: F401
        return
    except ImportError:
        pass
    try:
        from trn_agent_boot.trn_boot import _ntff_profile_via_ctypes
        hook = _ntff_profile_via_ctypes("/opt/axon/libaxon_pjrt.so")
    except Exception:
        hook = None
    mod = types.ModuleType("antenv.axon_hooks")
    mod._hook = hook
    mod.get_axon_ntff_profile_hook = lambda: mod._hook
    mod.set_axon_ntff_profile_hook = lambda h: setattr(mod, "_hook", h)
    sys.modules["antenv.axon_hooks"] = mod


def _get_program():
    if "nc" not in _BUILD_CACHE:
        _BUILD_CACHE["nc"] = _build_program()
    return _BUILD_CACHE["nc"]


def kernel(x, Wk, Wv, Wout, q_param, attn_scale, rpb_table):
    import ml_dtypes
    x = np.ascontiguousarray(np.asarray(x, dtype=np.float32)
                             .astype(ml_dtypes.bfloat16))
    wts = _host_weights(np.asarray(Wk), np.asarray(Wv), np.asarray(Wout),
                        np.asarray(q_param), np.asarray(attn_scale),
                        np.asarray(rpb_table))
    nc = _get_program()

    in_maps = []
    for c in range(NCORES):
        in_maps.append({
            "x": np.ascontiguousarray(x[c * BPC:(c + 1) * BPC]),
            "wkq": wts["wkq"], "wv": wts["wv"], "denk": wts["denk"],
            "kmat": wts["kmat"], "woutT": wts["woutT"], "ident": wts["ident"],
        })

    trace = bool(int(os.environ.get("KERNEL_TRACE", "0")))
    if trace:
        _install_ntff_shim()
    res = run_bass_kernel_spmd(nc, in_maps, core_ids=list(range(NCORES)),
                               trace=trace)
    _BUILD_CACHE["last_results"] = res

    out = np.empty((B, CS, HO, WO), np.float32)
    for c in range(NCORES):
        out[c * BPC:(c + 1) * BPC] = res.results[c]["out"]
    return out


# revision 20
# speedup vs baseline: 1.6497x; 1.0413x over previous
"""Trainium2 Bass kernel for nn_FusedKQnA (sparse attention with learned
queries + depthwise stride-2 conv aggregation).

Math restructuring (vs the reference):
  - k is never materialized: qkT = x^T @ (Wk @ QW) with QW the block-diagonal
    arrangement of the scaled learned queries -> one (128->32) matmul.
  - The global max subtractions inside the two exp() calls cancel exactly
    between numerator and denominator, so they are dropped.
  - The 1024-channel depthwise conv never materializes.  With
    r = 1/sum_den (computed as exp(-ln(den)), same ACT table set) define
        gamma[t,h,ij] = sum_q kern[t,q*8+h] * r[q*8+h,ij] * cost[n_t(ij),q*8+h]
    Then out_pre[(h,c),ij] = sum_t gamma[t,h,ij] * v[n_t(ij),(h,c)]  (256 ch)
    and out = Wout @ out_pre.
  - gamma's q-contraction + broadcast over the 32 channels of each head is a
    single small PE matmul per (tap, channel-chunk) with a one-hot*kern
    stationary operand; the tap accumulation is PSUM accumulation through
    identity matmuls.

Sharding: pure data parallel over batch: 16 batches -> 8 cores x 2.
"""

import os
from contextlib import ExitStack

import numpy as np

import concourse.bass as bass
import concourse.mybir as mybir
import concourse.tile as tile
from concourse import bacc
from concourse.bass_utils import run_bass_kernel_spmd

# Problem constants (hardcoded per spec nn_FusedKQnA_1726576854813)
N_Q, N_HEADS, KSIZE, STRIDE, PADDING = 4, 4, 3, 2, 1
B, C, H, W = 16, 128, 56, 56
HC = C // N_HEADS            # 32 head channels
HP = N_HEADS * STRIDE        # 8 effective heads
CS = C * STRIDE              # 256
G = N_Q * HP                 # 32 kernel groups
HO, WO = H // STRIDE, W // STRIDE   # 28, 28
NCORES = 8
BPC = B // NCORES            # batches per core

TAPS = [(di, dj) for di in (-1, 0, 1) for dj in (-1, 0, 1)]
N_STRIPS = 2                 # output rows split into strips of 14 (392 px)
ROWS_PER_STRIP = HO // N_STRIPS

F32 = mybir.dt.float32
BF16 = mybir.dt.bfloat16

_BUILD_CACHE = {}


def _host_weights(Wk, Wv, Wout, q_param, attn_scale, rpb_table):
    """Precompute all small weight tensors on the host."""
    q = q_param.reshape(N_Q, HP, HC).astype(np.float64) * (HC ** -0.5)
    QW = np.zeros((CS, G), np.float64)
    for qi in range(N_Q):
        for h in range(HP):
            QW[h * HC:(h + 1) * HC, qi * HP + h] = q[qi, h]
    wkq = (Wk.astype(np.float64) @ QW).astype(np.float32)        # (128, 32)

    rpb_exp = np.exp(rpb_table.astype(np.float64))               # (9, 32)
    kern_num = (rpb_exp * attn_scale.astype(np.float64))         # (9, 32)

    # denominator conv kernels as diagonal matmul weights: (9, G, G)
    denk = np.zeros((KSIZE * KSIZE, G, G), np.float32)
    for t in range(KSIZE * KSIZE):
        np.fill_diagonal(denk[t], rpb_exp[t])

    # gamma-broadcast stationary operands, stacked 3 taps per row-group for
    # tile_position packing: kmat[grp, ch][tau*32+g, m]
    kmat = np.zeros((3, 2, 3 * G, 128), np.float32)
    for t in range(KSIZE * KSIZE):
        grp, tau = divmod(t, 3)
        for ch in range(2):
            for g in range(G):
                h = g % HP
                if h // 4 == ch:
                    m0 = (h % 4) * HC
                    kmat[grp, ch, tau * G + g, m0:m0 + HC] = kern_num[t, g]

    woutT = np.ascontiguousarray(Wout.T.astype(np.float32))      # (256, 256) lhsT
    ident = np.eye(128, dtype=np.float32)
    import ml_dtypes
    return dict(wkq=wkq.astype(ml_dtypes.bfloat16), denk=denk, kmat=kmat,
                woutT=woutT, ident=ident,
                wv=np.ascontiguousarray(Wv.astype(ml_dtypes.bfloat16)))


def _build_program():
    """Build the Bass/Tile program once. Returns (nc, input_names)."""
    nc = bacc.Bacc("TRN2", target_bir_lowering=False, debug=False,
                   enable_asserts=False, num_devices=NCORES)

    x_d = nc.dram_tensor("x", [BPC, C, H, W], BF16, kind="ExternalInput").ap()
    wkq_d = nc.dram_tensor("wkq", [C, G], BF16, kind="ExternalInput").ap()
    wv_d = nc.dram_tensor("wv", [C, CS], BF16, kind="ExternalInput").ap()
    denk_d = nc.dram_tensor("denk", [9, G, G], F32, kind="ExternalInput").ap()
    kmat_d = nc.dram_tensor("kmat", [3, 2, 3 * G, 128], F32, kind="ExternalInput").ap()
    woutT_d = nc.dram_tensor("woutT", [CS, CS], F32, kind="ExternalInput").ap()
    ident_d = nc.dram_tensor("ident", [128, 128], F32, kind="ExternalInput").ap()
    out_d = nc.dram_tensor("out", [BPC, CS, HO, WO], F32, kind="ExternalOutput").ap()

    with tile.TileContext(nc) as tc, ExitStack() as ctx:
        _kernel_body(ctx, tc, out_d, x_d, wkq_d, wv_d, denk_d, kmat_d,
                     woutT_d, ident_d)

    _pin_act_tables()
    nc.compile()
    return nc


def _pin_act_tables():
    """Force one ACT table set (natural_log_exp_and_others) for Exp+Ln so the
    scheduler doesn't thrash table loads between them."""
    from concourse import hw_specs
    import concourse.bacc as bacc_mod
    if getattr(bacc_mod, "_act_tables_pinned", False):
        return
    orig = hw_specs.get_activation_tables

    def patched(arch):
        tabs = dict(orig(arch))
        keep = "natural_log_exp_and_others"
        for name in list(tabs):
            if name == keep:
                continue
            fns = tabs[name]
            if any(str(f).endswith((".Exp", ".Ln")) for f in fns):
                tabs[name] = type(fns)()
        return tabs

    bacc_mod.get_activation_tables = patched
    bacc_mod._act_tables_pinned = True


def _kernel_body(ctx, tc, out_d, x_d, wkq_d, wv_d, denk_d, kmat_d,
                 woutT_d, ident_d):
    nc = tc.nc
    PH, PW = H + 2, W + 2        # padded plane 58 x 58

    consts = ctx.enter_context(tc.tile_pool(name="consts", bufs=1))
    planes = ctx.enter_context(tc.tile_pool(name="planes", bufs=1))
    xpool = ctx.enter_context(tc.tile_pool(name="xpool", bufs=2))
    small = ctx.enter_context(tc.tile_pool(name="small", bufs=2))
    rcpool = ctx.enter_context(tc.tile_pool(name="rcpool", bufs=2))
    prod_pool = ctx.enter_context(tc.tile_pool(name="prod", bufs=4))
    opre_pool = ctx.enter_context(tc.tile_pool(name="opre", bufs=2))
    outs_pool = ctx.enter_context(tc.tile_pool(name="outs", bufs=4))

    ps = ctx.enter_context(tc.tile_pool(name="ps", bufs=2, space="PSUM"))

    # ---- constants into SBUF ----
    wkq_sb = consts.tile([C, G], BF16)
    nc.sync.dma_start(out=wkq_sb, in_=wkq_d)
    wv_sb = consts.tile([C, CS], BF16)
    nc.sync.dma_start(out=wv_sb, in_=wv_d)
    denk_sb = consts.tile([G, 9, G], BF16)
    nc.gpsimd.dma_start(out=denk_sb, in_=denk_d.transpose([1, 0, 2]))
    kmat_sb = consts.tile([3 * G, 3, 2, 128], BF16)
    nc.gpsimd.dma_start(out=kmat_sb, in_=kmat_d.transpose([2, 0, 1, 3]))

    woutT_sb = consts.tile([128, 2, CS], BF16)
    nc.gpsimd.dma_start(out=woutT_sb,
                        in_=woutT_d.rearrange("(kc k) m -> k kc m", kc=2))

    # ---- PE warm-up: ~3.5us of back-to-back matmuls during the x DMA so
    #      the HAM clock-gate opens (1.2 -> 2.4 GHz) before real work ----
    warm_sb = consts.tile([128, 128], BF16, name="warm_sb")
    nc.gpsimd.memset(warm_sb, 0.0)
    warm_ps = ps.tile([128, 128], F32, tag="gam", bufs=3, name="warm_ps")
    for _ in range(40):
        nc.tensor.matmul(warm_ps, warm_sb, warm_sb, start=True, stop=True)

    # ---- persistent padded planes (zero borders set once) ----
    cost_pl = [planes.tile([G, PH, PW], BF16, tag=f"cost{b}", name=f"cost_pl{b}") for b in range(BPC)]
    v_pl = [[planes.tile([128, PH, PW], BF16, tag=f"v{b}_{chn}", name=f"v_pl{b}_{chn}") for chn in range(2)]
            for b in range(BPC)]
    for pl in [cost_pl[b] for b in range(BPC)] + [v_pl[b][c] for b in range(BPC)
                                                 for c in range(2)]:
        # interior is fully overwritten every batch; only borders must be 0
        nc.gpsimd.memset(pl[:, 0, :], 0.0)
        nc.gpsimd.memset(pl[:, PH - 1, :], 0.0)
        nc.gpsimd.memset(pl[:, 1:PH - 1, 0], 0.0)
        nc.gpsimd.memset(pl[:, 1:PH - 1, PW - 1], 0.0)

    n_row_tiles = 7          # 56 rows in tiles of 8 -> matmul N=448
    RT = H // n_row_tiles    # 8 rows per tile

    def cview(b, di, dj):
        return cost_pl[b][:, 1 + di:1 + di + 2 * HO:2, 1 + dj:1 + dj + 2 * WO:2]

    # ---- phase A: load x, qkT + exp, v matmuls (both batches) ----
    x_sb = {}
    for b in range(BPC):
        x_sb[b] = xpool.tile([C, H, W], BF16, name=f"x_sb{b}")
        nc.sync.dma_start(out=x_sb[b], in_=x_d[b])
    for b in range(BPC):
        for rt in range(n_row_tiles):
            qk_ps = ps.tile([G, RT, W], F32, tag="mm", bufs=2, name="qk_ps")
            nc.tensor.matmul(qk_ps, wkq_sb, x_sb[b][:, rt * RT:(rt + 1) * RT, :],
                             start=True, stop=True)
            nc.scalar.activation(
                out=cost_pl[b][:, 1 + rt * RT:1 + (rt + 1) * RT, 1:1 + W],
                in_=qk_ps, func=mybir.ActivationFunctionType.Exp)
        for chn in range(2):
            for rt in range(n_row_tiles):
                v_ps = ps.tile([128, RT, W], F32, tag="mm", bufs=2, name="v_ps")
                nc.tensor.matmul(v_ps, wv_sb[:, chn * 128:(chn + 1) * 128],
                                 x_sb[b][:, rt * RT:(rt + 1) * RT, :],
                                 start=True, stop=True)
                if chn == 0:
                    nc.scalar.copy(
                        out=v_pl[b][chn][:, 1 + rt * RT:1 + (rt + 1) * RT,
                                         1:1 + W],
                        in_=v_ps)
                else:
                    nc.vector.tensor_copy(
                        out=v_pl[b][chn][:, 1 + rt * RT:1 + (rt + 1) * RT,
                                         1:1 + W],
                        in_=v_ps)

    # ---- phase B: den conv as diagonal matmuls on PE + r = exp(-ln(den)) ----
    r_sb = {}
    for b in range(BPC):
        den_ps = ps.tile([G, 2, 512], F32, tag="gam", bufs=3, name="den_ps")
        for s in range(N_STRIPS):
            r0 = s * ROWS_PER_STRIP
            dv = den_ps[:, s, :ROWS_PER_STRIP * WO].rearrange(
                "p (a c) -> p a c", a=ROWS_PER_STRIP)
            for t, (di, dj) in enumerate(TAPS):
                nc.tensor.matmul(
                    dv, denk_sb[:, t, :],
                    cview(b, di, dj)[:, r0:r0 + ROWS_PER_STRIP, :],
                    start=(t == 0), stop=(t == 8))
        lden = small.tile([G, 2, ROWS_PER_STRIP, WO], F32, tag="lden",
                          name="lden")
        dfull = den_ps[:, :, :ROWS_PER_STRIP * WO].rearrange(
            "p s (a c) -> p s a c", a=ROWS_PER_STRIP)
        nc.scalar.activation(out=lden, in_=dfull,
                             func=mybir.ActivationFunctionType.Ln)
        r_sb[b] = small.tile([G, 2, ROWS_PER_STRIP, WO], BF16, tag="rr",
                             name="rr")
        nc.scalar.activation(out=r_sb[b], in_=lden, scale=-1.0,
                             func=mybir.ActivationFunctionType.Exp)

    # ---- phase C: rc[t] = cost_t * r (on GpSimd; stacked 3 taps/tile) ----
    rc_st = {}
    for b in range(BPC):
        rc_st[b] = [rcpool.tile([3 * G, 2, ROWS_PER_STRIP, WO], BF16,
                                tag=f"rc{grp}", name=f"rc_st{grp}")
                    for grp in range(3)]
        for t, (di, dj) in enumerate(TAPS):
            grp, tau = divmod(t, 3)
            cvs = cview(b, di, dj).rearrange("p (s a) c -> p s a c", s=2)
            nc.gpsimd.tensor_mul(rc_st[b][grp][tau * G:(tau + 1) * G],
                                 cvs, r_sb[b])

    # ---- phases D+E per batch: packed gamma matmuls, products, DVE
    #      bf16 tree accumulation over taps ----
    opre_sb = {}
    for b in range(BPC):
        for chn in range(2):
            gsums = []
            for grp in range(3):
                gams = [ps.tile([128, 2, 512], F32, tag="gam", bufs=3,
                                name=f"gam_ps{tau}") for tau in range(3)]
                for s in range(N_STRIPS):
                    for tau in range(3):
                        gv = gams[tau][:, s, :ROWS_PER_STRIP * WO].rearrange(
                            "p (a c) -> p a c", a=ROWS_PER_STRIP)
                        nc.tensor.matmul(
                            gv, kmat_sb[tau * G:(tau + 1) * G, grp, chn, :],
                            rc_st[b][grp][tau * G:(tau + 1) * G, s],
                            start=True, stop=True,
                            tile_position=(tau * G, 0))
                ps_taps = []
                for tau in range(3):
                    t = grp * 3 + tau
                    di, dj = TAPS[t]
                    p_sb = prod_pool.tile([128, 2, ROWS_PER_STRIP, WO], BF16,
                                          tag="p", name=f"p{tau}")
                    gfull = gams[tau][:, :, :ROWS_PER_STRIP * WO].rearrange(
                        "p s (a c) -> p s a c", a=ROWS_PER_STRIP)
                    vv = v_pl[b][chn][:, 1 + di:1 + di + 2 * HO:2,
                                      1 + dj:1 + dj + 2 * WO:2].rearrange(
                        "p (s a) c -> p s a c", s=2)
                    nc.vector.tensor_mul(p_sb, gfull, vv)
                    ps_taps.append(p_sb)
                gs = prod_pool.tile([128, 2, ROWS_PER_STRIP, WO], BF16,
                                    tag="gs", name=f"gs{grp}")
                nc.vector.tensor_add(gs, ps_taps[0], ps_taps[1])
                nc.vector.tensor_add(gs, gs, ps_taps[2])
                gsums.append(gs)
            o_sb = opre_pool.tile([128, 2, ROWS_PER_STRIP, WO], BF16,
                                  tag=f"opre{chn}", name=f"opre{chn}")
            nc.vector.tensor_add(o_sb, gsums[0], gsums[1])
            nc.vector.tensor_add(o_sb, o_sb, gsums[2])
            for s in range(N_STRIPS):
                opre_sb[(b, chn, s)] = o_sb[:, s]

        for mo in range(2):
            for s in range(N_STRIPS):
                out_ps = ps.tile([128, ROWS_PER_STRIP, WO], F32, tag="mm",
                                 bufs=2, name="out_ps")
                for kc in range(2):
                    nc.tensor.matmul(out_ps,
                                     woutT_sb[:, kc, mo * 128:(mo + 1) * 128],
                                     opre_sb[(b, kc, s)],
                                     start=(kc == 0), stop=(kc == 1))
                o_final = outs_pool.tile([128, ROWS_PER_STRIP, WO], F32)
                nc.scalar.copy(out=o_final, in_=out_ps)
                nc.sync.dma_start(
                    out=out_d[b, mo * 128:(mo + 1) * 128,
                              s * ROWS_PER_STRIP:(s + 1) * ROWS_PER_STRIP, :],
                    in_=o_final)


def _install_ntff_shim():
    """bass_utils expects antenv.axon_hooks (absent in this checkout); shim it
    with the ctypes NTFF hook from trn_agent_boot so trace=True works."""
    import sys
    import types
    try:
        from antenv.axon_hooks import get_axon_ntff_profile_hook  # noqa: F401
        return
    except ImportError:
        pass
    try:
        from trn_agent_boot.trn_boot import _ntff_profile_via_ctypes
        hook = _ntff_profile_via_ctypes("/opt/axon/libaxon_pjrt.so")
    except Exception:
        hook = None
    mod = types.ModuleType("antenv.axon_hooks")
    mod._hook = hook
    mod.get_axon_ntff_profile_hook = lambda: mod._hook
    mod.set_axon_ntff_profile_hook = lambda h: setattr(mod, "_hook", h)
    sys.modules["antenv.axon_hooks"] = mod


def _get_program():
    if "nc" not in _BUILD_CACHE:
        _BUILD_CACHE["nc"] = _build_program()
    return _BUILD_CACHE["nc"]


def kernel(x, Wk, Wv, Wout, q_param, attn_scale, rpb_table):
    import ml_dtypes
    x = np.ascontiguousarray(np.asarray(x, dtype=np.float32)
                             .astype(ml_dtypes.bfloat16))
    wts = _host_weights(np.asarray(Wk), np.asarray(Wv), np.asarray(Wout),
                        np.asarray(q_param), np.asarray(attn_scale),
                        np.asarray(rpb_table))
    nc = _get_program()

    in_maps = []
    for c in range(NCORES):
        in_maps.append({
            "x": np.ascontiguousarray(x[c * BPC:(c + 1) * BPC]),
            "wkq": wts["wkq"], "wv": wts["wv"], "denk": wts["denk"],
            "kmat": wts["kmat"], "woutT": wts["woutT"], "ident": wts["ident"],
        })

    trace = bool(int(os.environ.get("KERNEL_TRACE", "0")))
    if trace:
        _install_ntff_shim()
    res = run_bass_kernel_spmd(nc, in_maps, core_ids=list(range(NCORES)),
                               trace=trace)
    _BUILD_CACHE["last_results"] = res

    out = np.empty((B, CS, HO, WO), np.float32)
    for c in range(NCORES):
        out[c * BPC:(c + 1) * BPC] = res.results[c]["out"]
    return out
